# revision 6
# baseline (speedup 1.0000x reference)
"""ConstituencyTreeLSTM Trainium2 kernel, v2.

Changes vs v1 baseline:
  - Leaf x-path in fp8 DoubleRow (accuracy-validated: rel ~1.46e-2 < 2e-2).
  - 32-node super-chunks: per-j activations merged along the node axis
    (same output slice -> same bias), tanh(c)/h/elementwise merged across
    the 4 output slices. ~240 Act instructions instead of ~460.
  - fx evacuated by DVE (per-slice psum->SBUF copies), f-gate preact =
    DVE add (psum + fx_t) -> tmp SBUF, act reads SBUF.
  - Partition-major DRAM packing for weights and inputs: every DMA is
    contiguous per partition (KB-sized descriptor runs, not 32-64B).
  - Tail (nodes 0..30 + 127) x-inputs SBUF-resident, loaded once.
  - Tile reuse: tanh(c) overwrites g_u, mul scratch overwrites g_i,
    split-h intermediate overwrites g_fl.
"""

import sys

sys.path.insert(0, "/opt/trn_rl_repo")

import numpy as np
import ml_dtypes

import concourse.bass as bass  # noqa: F401
import concourse.mybir as mybir
import concourse.tile as tile
from concourse import bacc
from concourse.bass_utils import run_bass_kernel_spmd

BF16 = ml_dtypes.bfloat16
FP8 = ml_dtypes.float8_e4m3
NCORES = 8
B, N, D = 256, 256, 512
BC = B // NCORES
NJ = 20
WSCALE = 16.0

# x-path blocks: 12 iou j-tiles + 4 fx j-tiles, 4 k-tiles each; iou js
# cohort-ordered (j = co, 4+co, 8+co) so the first DMA piece covers the
# first j-groups processed.
W_X_BLOCKS = [
    (kt, j) for co in range(4) for j in (co, 4 + co, 8 + co) for kt in range(4)
] + [(kt, j) for j in range(12, 16) for kt in range(4)]
WX_IDX = {p: i for i, p in enumerate(W_X_BLOCKS)}
NWX = len(W_X_BLOCKS)  # 64
NWX_IOU = 48

W_H_BLOCKS = []
for j in range(12):
    W_H_BLOCKS += [("L", kt, j) for kt in range(4)]
    W_H_BLOCKS += [("R", kt, j) for kt in range(4)]
for j in range(12, 16):
    W_H_BLOCKS += [("L", kt, j) for kt in range(4)]
for j in range(16, 20):
    W_H_BLOCKS += [("R", kt, j) for kt in range(4)]
WH_IDX = {p: i for i, p in enumerate(W_H_BLOCKS)}
NWH = len(W_H_BLOCKS)  # 128

# tail nodes resident in SBUF: 0..30 plus 127 at position 31
TAIL_POS = {n: n for n in range(31)}
TAIL_POS[127] = 31

IOU_ORDER = [0, 4, 8, 1, 5, 9, 2, 6, 10, 3, 7, 11]  # cohort order (DMA-friendly)

_compiled = {}


def _build_bass(reps=1):
    nc = bacc.Bacc("TRN2", target_bir_lowering=False, debug=False, num_devices=NCORES)

    f32 = mybir.dt.float32
    bf16 = mybir.dt.bfloat16
    fp8 = mybir.dt.float8e4
    DR = mybir.MatmulPerfMode.DoubleRow
    ACT = mybir.ActivationFunctionType

    # partition-major DRAM layouts (host pre-packed)
    xt8_d = nc.dram_tensor("xt8", [128, N, 4, BC], fp8, kind="ExternalInput")
    xtt_d = nc.dram_tensor("xtt", [128, 32, 4, BC], bf16, kind="ExternalInput")
    ident_d = nc.dram_tensor("ident", [128, 128], bf16, kind="ExternalInput")
    wx_d = nc.dram_tensor("wx", [128, NWX, 128], bf16, kind="ExternalInput")
    wx8_d = nc.dram_tensor("wx8", [128, NWX, 128], fp8, kind="ExternalInput")
    wh8_d = nc.dram_tensor("wh8", [128, NWH, 128], fp8, kind="ExternalInput")
    b2_d = nc.dram_tensor("b2", [128, NJ], f32, kind="ExternalInput")
    bleaf_d = nc.dram_tensor("bleaf", [128, NJ], f32, kind="ExternalInput")
    b1_d = nc.dram_tensor("b1", [128, NJ], f32, kind="ExternalInput")

    c0t = nc.dram_tensor("c0t", [D, BC], f32, kind="ExternalOutput")
    h0t = nc.dram_tensor("h0t", [D, BC], f32, kind="ExternalOutput")

    xt8_r = xt8_d.ap()
    c0t_r = c0t.ap().rearrange("(kt p) b -> p kt b", p=128)
    h0t_r = h0t.ap().rearrange("(kt p) b -> p kt b", p=128)

    with tile.TileContext(nc) as tc:
        import contextlib

        ctx = contextlib.ExitStack()
        with ctx:
            wpool = ctx.enter_context(tc.tile_pool(name="wpool", bufs=1))
            hpool = ctx.enter_context(tc.tile_pool(name="hpool", bufs=1))
            inpool = ctx.enter_context(tc.tile_pool(name="inpool", bufs=2))
            gpool = ctx.enter_context(tc.tile_pool(name="gpool", bufs=1))
            epool = ctx.enter_context(tc.tile_pool(name="epool", bufs=1))
            pspool = ctx.enter_context(tc.tile_pool(name="ps", bufs=4, space="PSUM"))

            # --- weights / biases / tail inputs (one-time) ----------------
            wx_sb = wpool.tile([128, NWX, 128], bf16, name="wxsb")
            wx8_sb = wpool.tile([128, NWX, 128], fp8, name="wx8")
            wh8_sb = wpool.tile([128, NWH, 128], fp8, name="wh8")
            b2_sb = wpool.tile([128, NJ], f32, name="b2sb")
            bleaf_sb = wpool.tile([128, NJ], f32, name="bleafsb")
            b1_sb = wpool.tile([128, NJ], f32, name="b1sb")
            xtt_sb = wpool.tile([128, 32, 4, BC], bf16, name="xttsb")
            ident_sb = wpool.tile([128, 128], bf16, name="identsb")
            b2s_sb = wpool.tile([128, NJ], bf16, name="b2ssb")
            b1s_sb = wpool.tile([128, NJ], bf16, name="b1ssb")

            nc.sync.dma_start(out=bleaf_sb[:], in_=bleaf_d.ap()[:])
            nc.sync.dma_start(out=b2_sb[:], in_=b2_d.ap()[:])
            nc.sync.dma_start(out=b1_sb[:], in_=b1_d.ap()[:])
            nc.sync.dma_start(out=ident_sb[:], in_=ident_d.ap()[:])
            # x16-scaled bf16 biases for the identity-matmul bias injection
            nc.vector.tensor_single_scalar(
                b2s_sb[:], b2_sb[:], WSCALE, mybir.AluOpType.mult
            )
            nc.vector.tensor_single_scalar(
                b1s_sb[:], b1_sb[:], WSCALE, mybir.AluOpType.mult
            )
            # order: fp8 iou x-blocks (leaves first), h weights (127/L6),
            # fp8 fx blocks (L6/L5), bf16 wx + biases + tail x.
            for s in range(0, NWX_IOU, 12):
                nc.gpsimd.dma_start(
                    out=wx8_sb[:, s : s + 12, :], in_=wx8_d.ap()[:, s : s + 12, :]
                )
            nc.gpsimd.dma_start(out=xtt_sb[:], in_=xtt_d.ap()[:])
            for s in range(0, NWH, 32):
                nc.gpsimd.dma_start(
                    out=wh8_sb[:, s : s + 32, :], in_=wh8_d.ap()[:, s : s + 32, :]
                )
            nc.gpsimd.dma_start(
                out=wx8_sb[:, NWX_IOU:, :], in_=wx8_d.ap()[:, NWX_IOU:, :]
            )
            nc.gpsimd.dma_start(out=wx_sb[:], in_=wx_d.ap()[:])

            def process(
                nodes,
                has_l,
                has_r,
                bias_sb,
                child_h,  # list[(tile, base)] or None
                out_h,  # list[(tile, base)] or None (root)
                child_c=None,
                out_c=None,
                x8=False,
                bias_s_sb=None,
            ):
                a, b_ = nodes.start, nodes.stop
                K = b_ - a
                SUB = (K + 15) // 16
                ks = [min(16, K - 16 * s) for s in range(SUB)]
                to_out = out_h is None
                dt_g = f32 if to_out else bf16
                have_f = has_l or has_r

                # x input: fp8 streamed tile, or resident bf16 tail slice
                if x8:
                    xt_t = inpool.tile([128, K, 4, BC], fp8, name="xt8_t")
                    nc.sync.dma_start(out=xt_t[:], in_=xt8_r[:, a:b_, :, :])
                    xv = xt_t
                else:
                    p0 = TAIL_POS[a]
                    xv = xtt_sb[:, p0 : p0 + K, :, :]

                def x_insts(s, j):
                    n0 = 16 * s
                    n1 = n0 + ks[s]
                    jx = j - 4 if j >= 16 else j  # fR's x-part is fx too
                    if x8:
                        i0 = WX_IDX[(0, jx)]
                        return [
                            (
                                wx8_sb[:, i0 + kk : i0 + kk + 2, :],
                                xv[:, n0:n1, kk : kk + 2, :].rearrange(
                                    "p n kt b -> p kt n b"
                                ),
                                DR,
                            )
                            for kk in (0, 2)
                        ]
                    return [
                        (
                            wx_sb[:, WX_IDX[(kk, jx)], :],
                            xv[:, n0:n1, kk, :],
                            None,
                        )
                        for kk in range(4)
                    ]

                def h_insts(s, j):
                    if child_h is None:
                        return []
                    ch_base = child_h[0][1]
                    n0, n1 = a + 16 * s, a + 16 * s + ks[s]
                    sl0 = 2 * n0 + 1 - ch_base
                    kk = n1 - n0

                    def nsl(off):
                        s0 = sl0 + off
                        if kk == 1:
                            return slice(s0, s0 + 1)
                        return slice(s0, s0 + 2 * kk - 1, 2)

                    sides = []
                    if has_l and j < 16:
                        sides.append(("L", 0))
                    if has_r and (j < 12 or 16 <= j):
                        sides.append(("R", 1))
                    out = []
                    for side, off in sides:
                        i0 = WH_IDX[(side, 0, j)]
                        for ct, _ in child_h:
                            for kta in (0, 2):
                                out.append(
                                    (
                                        wh8_sb[:, i0 + kta : i0 + kta + 2, :],
                                        ct[:, nsl(off), kta : kta + 2, :].rearrange(
                                            "p n kt b -> p kt n b"
                                        ),
                                        DR,
                                    )
                                )
                    return out

                def mm_group(ps, j, x_part=True, h_part=True):
                    # x phase for all subs first, then h phase: stalled
                    # h-matmuls sit behind ready x-work, not in front of it
                    # (PE dependency wait-queue is only 4 deep). Each sub's
                    # region is its own bank, so per-sub start flags are safe.
                    phases = []
                    for s in range(SUB):
                        xi = x_insts(s, j) if x_part else []
                        hi = h_insts(s, j) if h_part else []
                        phases.append((s, xi, hi))
                    for pi in range(2):
                        for s, xi, hi in phases:
                            psv = ps[:, s, : ks[s], :]
                            insts = xi if pi == 0 else hi
                            if not insts:
                                continue
                            first = pi == 0 or not xi
                            last = pi == 1 or not hi
                            for m, (w_ap, rhs, pm) in enumerate(insts):
                                kw = {} if pm is None else {"perf_mode": pm}
                                nc.tensor.matmul(
                                    psv,
                                    w_ap,
                                    rhs,
                                    start=(first and m == 0),
                                    stop=(last and m == len(insts) - 1),
                                    **kw,
                                )

                g_i = gpool.tile([128, K, 4, BC], dt_g, name="g_i", bufs=2)
                g_o = gpool.tile([128, K, 4, BC], dt_g, name="g_o", bufs=1)
                g_u = gpool.tile([128, K, 4, BC], dt_g, name="g_u", bufs=2)
                if have_f:
                    # fx shared between fL/fR: computed once, applied by DVE
                    # (PE is the HW bottleneck; DVE has slack)
                    fx_t = gpool.tile([128, K, 4, BC], dt_g, name="fx_t")
                if has_l:
                    g_fl = gpool.tile([128, K, 4, BC], dt_g, name="g_fl")
                if has_r:
                    g_fr = gpool.tile([128, K, 4, BC], dt_g, name="g_fr")

                def gv(t, sl):
                    """[p, SUB, 16, b] view of gate tile t's output-slice sl."""
                    if SUB == 1:
                        return t[:, :K, sl, :]
                    return t[:].rearrange("p (s n) kt b -> p s n kt b", s=SUB)[
                        :, :, :, sl, :
                    ]

                def psv_all(ps):
                    if SUB == 1:
                        return ps[:, 0, :K, :]
                    return ps[:]

                if out_c is not None:
                    oc_t, oc_base = out_c
                    c_t = oc_t[:, a - oc_base : b_ - oc_base, :, :]
                else:
                    c_t = epool.tile([128, K, 4, BC], dt_g, name="c_t")[:]
                if to_out:
                    h_t = epool.tile([128, K, 4, BC], dt_g, name="h_t")

                if child_c is not None:
                    cc_t, cc_base = child_c
                    cs0 = 2 * a + 1 - cc_base
                    if has_l:
                        if K == 1:
                            cl_t = cc_t[:, cs0 : cs0 + 1, :, :]
                        else:
                            cl_t = cc_t[:, cs0 : cs0 + 2 * K - 1 : 2, :, :]
                    if has_r:
                        if K == 1:
                            cr_t = cc_t[:, cs0 + 1 : cs0 + 2, :, :]
                        else:
                            cr_t = cc_t[:, cs0 + 1 : cs0 + 2 * K : 2, :, :]

                def gate_act(dst, sl, j, func):
                    ps = pspool.tile([128, 2, 16, BC], f32, name="ps")
                    mm_group(ps, j)
                    nc.scalar.activation(
                        out=gv(dst, sl),
                        in_=psv_all(ps),
                        func=func,
                        bias=bias_sb[:, j : j + 1],
                        scale=1.0 / WSCALE,
                    )

                if K <= 8:
                    # ---- gate-merged tail path: one act per gate, bias
                    # injected into PSUM via identity-matmul with a
                    # stride-0 broadcast rhs (bias pre-scaled x16).
                    # Two-phase emission: ready bias/x matmuls for several
                    # groups first, stalled h matmuls after, so the 4-deep
                    # PE wait-queue never hides ready work. start= is set
                    # only on the first matmul of each PSUM bank (slices
                    # share banks at small K; start clears the whole bank's
                    # has_written bits). -------------------------------------
                    spb = max(1, 512 // (K * BC))  # slices per psum bank

                    def mm_tail(ps, j0, phase, bias_mm=True, x_part=True,
                                h_part=True):
                        for sl in range(4):
                            j = j0 + sl
                            a_insts = []
                            if bias_mm:
                                a_insts.append(
                                    (
                                        ident_sb[:],
                                        bias_s_sb[:, j : j + 1].broadcast_to(
                                            [128, K * BC]
                                        ),
                                        None,
                                    )
                                )
                            if x_part:
                                a_insts += x_insts(0, j)
                            b_insts = h_insts(0, j) if h_part else []
                            insts = a_insts if phase == 0 else b_insts
                            if not insts:
                                continue
                            first_of_slice = phase == 0 or not a_insts
                            last_of_slice = phase == 1 or not b_insts
                            for m, (w_ap, rhs, pm) in enumerate(insts):
                                kw = {} if pm is None else {"perf_mode": pm}
                                st = (
                                    first_of_slice and m == 0 and sl % spb == 0
                                )
                                sp = last_of_slice and m == len(insts) - 1
                                nc.tensor.matmul(
                                    ps[:, sl, :, :], w_ap, rhs,
                                    start=st, stop=sp, **kw,
                                )

                    def gview(t):
                        return t[:, :K, :, :].rearrange("p n s b -> p s n b")

                    def ps4():
                        return pspool.tile([128, 4, K, BC], f32, name="ps")

                    ps_fx = ps4() if have_f else None
                    ps_fl = ps4() if has_l else None
                    ps_fr = ps4() if has_r else None
                    ps_i, ps_u = ps4(), ps4()
                    # phase A: all ready (bias + x) work
                    if have_f:
                        mm_tail(ps_fx, 12, 0, bias_mm=False, h_part=False)
                    if has_l:
                        mm_tail(ps_fl, 12, 0, x_part=False)
                    if has_r:
                        mm_tail(ps_fr, 16, 0, x_part=False)
                    mm_tail(ps_i, 0, 0)
                    mm_tail(ps_u, 8, 0)
                    if have_f:
                        nc.vector.tensor_copy(gview(fx_t), ps_fx[:])
                    # phase B: h accumulation
                    if has_l:
                        mm_tail(ps_fl, 12, 1, x_part=False)
                    if has_r:
                        mm_tail(ps_fr, 16, 1, x_part=False)
                    mm_tail(ps_i, 0, 1)
                    mm_tail(ps_u, 8, 1)
                    nc.scalar.activation(
                        out=gview(g_i), in_=ps_i[:], func=ACT.Sigmoid,
                        scale=1.0 / WSCALE,
                    )
                    nc.scalar.activation(
                        out=gview(g_u), in_=ps_u[:], func=ACT.Tanh,
                        scale=1.0 / WSCALE,
                    )
                    if has_l:
                        nc.vector.tensor_add(gview(g_fl), ps_fl[:], gview(fx_t))
                        nc.scalar.activation(
                            out=gview(g_fl), in_=gview(g_fl),
                            func=ACT.Sigmoid, scale=1.0 / WSCALE,
                        )
                    if has_r:
                        nc.vector.tensor_add(gview(g_fr), ps_fr[:], gview(fx_t))
                        nc.scalar.activation(
                            out=gview(g_fr), in_=gview(g_fr),
                            func=ACT.Sigmoid, scale=1.0 / WSCALE,
                        )
                    ps_o = ps4()
                    mm_tail(ps_o, 4, 0)
                    mm_tail(ps_o, 4, 1)
                    nc.scalar.activation(
                        out=gview(g_o), in_=ps_o[:], func=ACT.Sigmoid,
                        scale=1.0 / WSCALE,
                    )

                    gi = g_i[:, :K, :, :]
                    go = g_o[:, :K, :, :]
                    gu = g_u[:, :K, :, :]
                    nc.vector.tensor_mul(c_t, gi, gu)
                    if has_l:
                        nc.vector.tensor_mul(gi, g_fl[:, :K, :, :], cl_t)
                        nc.vector.tensor_add(c_t, c_t, gi)
                    if has_r:
                        nc.vector.tensor_mul(gi, g_fr[:, :K, :, :], cr_t)
                        nc.vector.tensor_add(c_t, c_t, gi)
                    nc.scalar.activation(out=gu, in_=c_t, func=ACT.Tanh)
                    if to_out:
                        nc.vector.tensor_mul(h_t[:], go, gu)
                        nc.sync.dma_start(out=c0t_r[:], in_=c_t[:, 0, :, :])
                        nc.sync.dma_start(out=h0t_r[:], in_=h_t[:, 0, :, :])
                    elif len(out_h) == 1:
                        oh_t, oh_base = out_h[0]
                        nc.vector.tensor_mul(
                            oh_t[:, a - oh_base : b_ - oh_base, :, :], go, gu
                        )
                    else:
                        hbf = g_fl[:, :K, :, :]
                        nc.vector.tensor_mul(hbf, go, gu)
                        hsl = slice(a - out_h[0][1], b_ - out_h[0][1])
                        h8s = out_h[0][0][:, hsl, :, :]
                        nc.vector.tensor_copy(h8s, hbf)
                        nc.vector.tensor_sub(out_h[1][0][:, hsl, :, :], hbf, h8s)
                    return

                # two ktpair halves: groups emitted in consumption order
                # {i,u} -> {fx} -> {fL,fR} -> {o}, then this half's
                # elementwise + tanh + h, so DoubleRow consumers of child h
                # at the next level unblock per-ktpair.
                for hp in (0, 2):
                    sls = (hp, hp + 1)
                    for sl in sls:
                        gate_act(g_i, sl, sl, ACT.Sigmoid)
                        gate_act(g_u, sl, 8 + sl, ACT.Tanh)
                    if have_f:
                        for sl in sls:
                            ps_fx = pspool.tile([128, 2, 16, BC], f32, name="ps")
                            mm_group(ps_fx, 12 + sl, h_part=False)
                            nc.vector.tensor_copy(gv(fx_t, sl), psv_all(ps_fx))
                        for side_j, g_f in (
                            (12, g_fl if has_l else None),
                            (16, g_fr if has_r else None),
                        ):
                            if g_f is None:
                                continue
                            for sl in sls:
                                j = side_j + sl
                                ps = pspool.tile([128, 2, 16, BC], f32, name="ps")
                                mm_group(ps, j, x_part=False)
                                nc.vector.tensor_add(
                                    gv(g_f, sl), psv_all(ps), gv(fx_t, sl)
                                )
                                nc.scalar.activation(
                                    out=gv(g_f, sl),
                                    in_=gv(g_f, sl),
                                    func=ACT.Sigmoid,
                                    bias=bias_sb[:, j : j + 1],
                                    scale=1.0 / WSCALE,
                                )
                    for sl in sls:
                        gate_act(g_o, sl, 4 + sl, ACT.Sigmoid)

                    # --- elementwise for this ktpair half ----------------
                    h2 = slice(hp, hp + 2)
                    ch = c_t[:, :, h2, :]
                    gi = g_i[:, :K, h2, :]
                    go = g_o[:, :K, h2, :]
                    gu = g_u[:, :K, h2, :]
                    nc.vector.tensor_mul(ch, gi, gu)
                    if has_l:
                        nc.vector.tensor_mul(gi, g_fl[:, :K, h2, :], cl_t[:, :, h2, :])
                        nc.vector.tensor_add(ch, ch, gi)
                    if has_r:
                        nc.vector.tensor_mul(gi, g_fr[:, :K, h2, :], cr_t[:, :, h2, :])
                        nc.vector.tensor_add(ch, ch, gi)
                    # tanh(c) -> reuse g_u
                    nc.scalar.activation(out=gu, in_=ch, func=ACT.Tanh)
                    if to_out:
                        nc.vector.tensor_mul(h_t[:, :, h2, :], go, gu)
                    elif len(out_h) == 1:
                        oh_t, oh_base = out_h[0]
                        nc.vector.tensor_mul(
                            oh_t[:, a - oh_base : b_ - oh_base, h2, :], go, gu
                        )
                    else:
                        # split-h: hbf reuses g_fl (consumed above)
                        hbf = g_fl[:, :K, h2, :]
                        nc.vector.tensor_mul(hbf, go, gu)
                        hsl = slice(a - out_h[0][1], b_ - out_h[0][1])
                        h8s = out_h[0][0][:, hsl, h2, :]
                        nc.vector.tensor_copy(h8s, hbf)
                        nc.vector.tensor_sub(
                            out_h[1][0][:, hsl, h2, :], hbf, h8s
                        )

                if to_out:
                    nc.sync.dma_start(out=c0t_r[:], in_=c_t[:, 0, :, :])
                    nc.sync.dma_start(out=h0t_r[:], in_=h_t[:, 0, :, :])

            # h storage: plain fp8 levels 4..7; split fp8 pair levels 1..3.
            # c: fp8 at level 7, bf16 below.
            H_SPLIT_LVLS = (3, 2, 1)

            for _rep in range(reps):
                leafc_h = hpool.tile([128, 129, 4, BC], fp8, name="h_leafc")
                leafc_c = hpool.tile([128, 129, 4, BC], fp8, name="c_leafc")
                lvl_h = {7: [(leafc_h, 127)]}
                lvl_c = {7: (leafc_c, 127)}
                for lvl in range(6, 0, -1):
                    base = 2**lvl - 1
                    if lvl in H_SPLIT_LVLS:
                        t8 = hpool.tile([128, 2**lvl, 4, BC], fp8, name=f"h_{lvl}")
                        r8 = hpool.tile([128, 2**lvl, 4, BC], fp8, name=f"hr_{lvl}")
                        lvl_h[lvl] = [(t8, base), (r8, base)]
                    else:
                        t = hpool.tile([128, 2**lvl, 4, BC], fp8, name=f"h_{lvl}")
                        lvl_h[lvl] = [(t, base)]
                    t = hpool.tile([128, 2**lvl, 4, BC], bf16, name=f"c_{lvl}")
                    lvl_c[lvl] = (t, base)

                # leaves in 32-node super-chunks; the one holding node 255
                # first so node 127's serial chain hides behind the rest.
                for s4 in (224, 128):
                    process(
                        range(s4, s4 + 32), False, False, bleaf_sb, None,
                        lvl_h[7], out_c=lvl_c[7], x8=True,
                    )
                    if s4 == 224:
                        process(
                            range(127, 128), True, False, b1_sb, lvl_h[7],
                            lvl_h[7], child_c=lvl_c[7], out_c=lvl_c[7],
                            bias_s_sb=b1s_sb,
                        )
                for s4 in (160, 192):
                    process(
                        range(s4, s4 + 32), False, False, bleaf_sb, None,
                        lvl_h[7], out_c=lvl_c[7], x8=True,
                    )
                # L6: B-half (63..94, needs node 127 + leaves 128..190) after
                # A-half? A (95..126) needs leaves 191..254 -> do B first?
                # B needs 127..190 (ready after leaves 128..191); A needs
                # 191..254 (ready after all leaves). Emit B then A.
                process(
                    range(63, 95), True, True, b2_sb, lvl_h[7], lvl_h[6],
                    child_c=lvl_c[7], out_c=lvl_c[6], x8=True,
                )
                process(
                    range(95, 127), True, True, b2_sb, lvl_h[7], lvl_h[6],
                    child_c=lvl_c[7], out_c=lvl_c[6], x8=True,
                )
                # L5 (one 32-node super-chunk), then L4..L1
                process(
                    range(31, 63), True, True, b2_sb, lvl_h[6], lvl_h[5],
                    child_c=lvl_c[6], out_c=lvl_c[5], x8=True,
                )
                # L4 as two 8-node gate-merged chunks (pipeline each other)
                for a4 in (15, 23):
                    process(
                        range(a4, a4 + 8), True, True, b2_sb, lvl_h[5],
                        lvl_h[4], child_c=lvl_c[5], out_c=lvl_c[4],
                        bias_s_sb=b2s_sb,
                    )
                for lvl in range(3, 0, -1):
                    process(
                        range(2**lvl - 1, 2 ** (lvl + 1) - 1), True, True,
                        b2_sb, lvl_h[lvl + 1], lvl_h[lvl],
                        child_c=lvl_c[lvl + 1], out_c=lvl_c[lvl],
                        bias_s_sb=b2s_sb,
                    )
                process(
                    range(0, 1), True, True, b2_sb, lvl_h[1], None,
                    child_c=lvl_c[1], bias_s_sb=b2s_sb,
                )

    nc.compile()
    return nc


def _expected_tree():
    left = np.array([2 * i + 1 if 2 * i + 1 < N else 0 for i in range(N)], np.int32)
    right = np.array([2 * i + 2 if 2 * i + 2 < N else 0 for i in range(N)], np.int32)
    nch = np.array(
        [int(2 * i + 1 < N) + int(2 * i + 2 < N) for i in range(N)], np.int32
    )
    return left, right, nch


def pack_w(W_ioux, W_fx, W_iouhL, W_fhL, W_iouhR, W_fhR):
    """Returns (wx bf16, wx8 fp8, wh8 fp8), partition-major [128, blk, 128]."""
    s = WSCALE
    WxT = np.asarray(W_ioux, np.float32).T * s
    WfxT = np.asarray(W_fx, np.float32).T * s
    wx = np.empty((NWX, 128, 128), np.float32)
    for i, (kt, j) in enumerate(W_X_BLOCKS):
        src = WxT if j < 12 else WfxT
        jj = j if j < 12 else j - 12
        wx[i] = src[kt * 128 : (kt + 1) * 128, jj * 128 : (jj + 1) * 128]

    WhT = {
        "L": (np.asarray(W_iouhL, np.float32).T * s,
              np.asarray(W_fhL, np.float32).T * s),
        "R": (np.asarray(W_iouhR, np.float32).T * s,
              np.asarray(W_fhR, np.float32).T * s),
    }
    wh = np.empty((NWH, 128, 128), np.float32)
    for i, (side, kt, j) in enumerate(W_H_BLOCKS):
        iou_m, f_m = WhT[side]
        if j < 12:
            wh[i] = iou_m[kt * 128 : (kt + 1) * 128, j * 128 : (j + 1) * 128]
        else:
            jj = (j - 12) if j < 16 else (j - 16)
            wh[i] = f_m[kt * 128 : (kt + 1) * 128, jj * 128 : (jj + 1) * 128]

    wx_pm = np.ascontiguousarray(wx.transpose(1, 0, 2))  # [128, blk, 128]
    wh_pm = np.ascontiguousarray(wh.transpose(1, 0, 2))
    return wx_pm.astype(BF16), wx_pm.astype(FP8), wh_pm.astype(FP8)


def pack_biases(b_ioux, b_iouh, b_iouhL, b_iouhR, b_fx, b_fhL, b_fhR):
    def pack(vec):
        return np.ascontiguousarray(np.asarray(vec, np.float32).reshape(NJ, 128).T)

    z = np.zeros(512, np.float32)
    b2 = pack(np.concatenate([b_ioux + b_iouhL + b_iouhR, b_fx + b_fhL, b_fx + b_fhR]))
    bleaf = pack(np.concatenate([b_ioux + b_iouh, z, z]))
    b1 = pack(np.concatenate([b_ioux + b_iouhL, b_fx + b_fhL, z]))
    return b2, bleaf, b1


def pack_x_all(inputs):
    """inputs: [B, N, D] f32 -> per-core (xt8 [128,N,4,BC] fp8,
    xtt [128,32,4,BC] bf16) lists, one vectorized pass."""
    x = inputs.reshape(NCORES, BC, N, 4, 128)
    xt = np.ascontiguousarray(x.transpose(0, 4, 2, 3, 1))  # [C,128,N,4,BC]
    xt8 = xt.astype(FP8)
    tail = np.empty((NCORES, 128, 32, 4, BC), np.float32)
    tail[:, :, :31] = xt[:, :, :31]
    tail[:, :, 31] = xt[:, :, 127]
    tail = tail.astype(BF16)
    return [xt8[c] for c in range(NCORES)], [tail[c] for c in range(NCORES)]


class _Runner:
    """jit once per nc; reuse the executable across calls."""

    def __init__(self, nc, n_cores):
        import jax
        from concourse import bass2jax
        from concourse.bass2jax import _bass_exec_p, install_neuronx_cc_hook

        install_neuronx_cc_hook()
        self.nc = nc
        self.n_cores = n_cores
        partition_name = (
            nc.partition_id_tensor.name if nc.partition_id_tensor else None
        )
        in_names, out_names, out_avals, zero_outs = [], [], [], []
        for alloc in nc.m.functions[0].allocations:
            if not isinstance(alloc, mybir.MemoryLocationSet):
                continue
            name = alloc.memorylocations[0].name
            if alloc.kind == "ExternalInput":
                if name != partition_name:
                    in_names.append(name)
            elif alloc.kind == "ExternalOutput":
                out_names.append(name)
                shape = tuple(alloc.tensor_shape)
                dtype = mybir.dt.np(alloc.dtype)
                out_avals.append(jax.core.ShapedArray(shape, dtype))
                zero_outs.append(np.zeros(shape, dtype))
        self.in_names = in_names
        self.out_names = out_names
        self.zero_outs = zero_outs
        n_params = len(in_names)
        all_in = in_names + out_names
        if partition_name is not None:
            all_in.append(partition_name)

        def _body(*args):
            operands = list(args)
            if partition_name is not None:
                operands.append(bass2jax.partition_id_tensor())
            outs = _bass_exec_p.bind(
                *operands,
                out_avals=tuple(out_avals),
                in_names=tuple(all_in),
                out_names=tuple(out_names),
                lowering_input_output_aliases=(),
                sim_require_finite=True,
                sim_require_nnan=True,
                nc=nc,
            )
            return tuple(outs)

        if n_cores == 1:
            self.fn = jax.jit(_body, keep_unused=True)
        else:
            from jax.sharding import Mesh, PartitionSpec
            from jax.experimental.shard_map import shard_map

            devices = jax.devices()[:n_cores]
            mesh = Mesh(np.asarray(devices), ("core",))
            n_out = len(out_names)
            self.fn = jax.jit(
                shard_map(
                    _body,
                    mesh=mesh,
                    in_specs=(PartitionSpec("core"),) * (n_params + n_out),
                    out_specs=(PartitionSpec("core"),) * n_out,
                    check_rep=False,
                ),
                keep_unused=True,
            )

    def __call__(self, in_maps):
        import jax

        n = self.n_cores
        if n == 1:
            args = [np.asarray(in_maps[0][k]) for k in self.in_names]
            args += [np.zeros_like(z) for z in self.zero_outs]
            outs = self.fn(*args)
            jax.block_until_ready(outs)
            return [{k: np.asarray(outs[i]) for i, k in enumerate(self.out_names)}]
        args = [
            np.concatenate([np.asarray(m[k]) for m in in_maps], axis=0)
            for k in self.in_names
        ]
        args += [
            np.zeros((n * z.shape[0], *z.shape[1:]), z.dtype) for z in self.zero_outs
        ]
        outs = self.fn(*args)
        jax.block_until_ready(outs)
        res = []
        for c in range(n):
            d = {}
            for i, k in enumerate(self.out_names):
                full = np.asarray(outs[i])
                per = full.shape[0] // n
                d[k] = full[c * per : (c + 1) * per]
            res.append(d)
        return res


def _make_in_maps(inputs, weights_args):
    wx, wx8, wh8 = pack_w(*weights_args[:6])
    b2, bleaf, b1 = pack_biases(*weights_args[6:])
    inputs = np.asarray(inputs, np.float32)
    ident = np.eye(128, dtype=BF16)
    xt8s, xtts = pack_x_all(inputs)
    in_maps = []
    for c in range(NCORES):
        in_maps.append(
            {"xt8": xt8s[c], "xtt": xtts[c], "wx": wx, "wx8": wx8, "wh8": wh8,
             "b2": b2, "bleaf": bleaf, "b1": b1, "ident": ident}
        )
    return in_maps


def kernel(
    inputs,
    W_ioux, b_ioux, W_iouh, b_iouh, W_iouhL, b_iouhL, W_iouhR, b_iouhR,
    W_fx, b_fx, W_fh, b_fh, W_fhL, b_fhL, W_fhR, b_fhR,
    left_idx, right_idx, num_children,
):
    el, er, en = _expected_tree()
    assert np.array_equal(np.asarray(left_idx), el), "unexpected tree structure"
    assert np.array_equal(np.asarray(right_idx), er), "unexpected tree structure"
    assert np.array_equal(np.asarray(num_children), en), "unexpected tree structure"

    weights_args = (W_ioux, W_fx, W_iouhL, W_fhL, W_iouhR, W_fhR,
                    b_ioux, b_iouh, b_iouhL, b_iouhR, b_fx, b_fhL, b_fhR)
    in_maps = _make_in_maps(inputs, weights_args)

    if "nc" not in _compiled:
        _compiled["nc"] = _build_bass()
    nc = _compiled["nc"]
    if "runner" not in _compiled:
        _compiled["runner"] = _Runner(nc, NCORES)
    res = _compiled["runner"](in_maps)
    _compiled["last_res"] = res

    c_full = np.empty((B, D), np.float32)
    h_full = np.empty((B, D), np.float32)
    for c in range(NCORES):
        c_full[c * BC : (c + 1) * BC] = res[c]["c0t"].T
        h_full[c * BC : (c + 1) * BC] = res[c]["h0t"].T
    return c_full, h_full


# revision 8
# speedup vs baseline: 1.1198x; 1.1198x over previous
"""ConstituencyTreeLSTM Trainium2 kernel, v2.

Changes vs v1 baseline:
  - Leaf x-path in fp8 DoubleRow (accuracy-validated: rel ~1.46e-2 < 2e-2).
  - 32-node super-chunks: per-j activations merged along the node axis
    (same output slice -> same bias), tanh(c)/h/elementwise merged across
    the 4 output slices. ~240 Act instructions instead of ~460.
  - fx recomputed into each f-gate side's PSUM group (A/B-benched faster
    on HW than sharing it via DVE copy+add, despite the extra matmuls).
  - Partition-major DRAM packing for weights and inputs: every DMA is
    contiguous per partition (KB-sized descriptor runs, not 32-64B).
  - Tail (nodes 0..30 + 127) x-inputs SBUF-resident, loaded once.
  - Tile reuse: tanh(c) overwrites g_u, mul scratch overwrites g_i,
    split-h intermediate overwrites g_fl.
"""

import sys

sys.path.insert(0, "/opt/trn_rl_repo")

import numpy as np
import ml_dtypes

import concourse.bass as bass  # noqa: F401
import concourse.mybir as mybir
import concourse.tile as tile
from concourse import bacc
from concourse.bass_utils import run_bass_kernel_spmd

BF16 = ml_dtypes.bfloat16
FP8 = ml_dtypes.float8_e4m3
NCORES = 8
B, N, D = 256, 256, 512
BC = B // NCORES
NJ = 20
WSCALE = 16.0

# x-path blocks: 12 iou j-tiles + 4 fx j-tiles, 4 k-tiles each; iou js
# cohort-ordered (j = co, 4+co, 8+co) so the first DMA piece covers the
# first j-groups processed.
W_X_BLOCKS = [
    (kt, j) for co in range(4) for j in (co, 4 + co, 8 + co) for kt in range(4)
] + [(kt, j) for j in range(12, 16) for kt in range(4)]
WX_IDX = {p: i for i, p in enumerate(W_X_BLOCKS)}
NWX = len(W_X_BLOCKS)  # 64
NWX_IOU = 48

W_H_BLOCKS = []
for j in range(12):
    W_H_BLOCKS += [("L", kt, j) for kt in range(4)]
    W_H_BLOCKS += [("R", kt, j) for kt in range(4)]
for j in range(12, 16):
    W_H_BLOCKS += [("L", kt, j) for kt in range(4)]
for j in range(16, 20):
    W_H_BLOCKS += [("R", kt, j) for kt in range(4)]
WH_IDX = {p: i for i, p in enumerate(W_H_BLOCKS)}
NWH = len(W_H_BLOCKS)  # 128

# tail nodes resident in SBUF: 0..30 plus 127 at position 31
TAIL_POS = {n: n for n in range(31)}
TAIL_POS[127] = 31

IOU_ORDER = [0, 4, 8, 1, 5, 9, 2, 6, 10, 3, 7, 11]  # cohort order (DMA-friendly)

_compiled = {}


def _build_bass(reps=1):
    nc = bacc.Bacc("TRN2", target_bir_lowering=False, debug=False, num_devices=NCORES)

    f32 = mybir.dt.float32
    bf16 = mybir.dt.bfloat16
    fp8 = mybir.dt.float8e4
    DR = mybir.MatmulPerfMode.DoubleRow
    ACT = mybir.ActivationFunctionType

    # partition-major DRAM layouts (host pre-packed)
    xt8_d = nc.dram_tensor("xt8", [128, N, 4, BC], fp8, kind="ExternalInput")
    xtt_d = nc.dram_tensor("xtt", [128, 32, 4, BC], bf16, kind="ExternalInput")
    ident_d = nc.dram_tensor("ident", [128, 128], bf16, kind="ExternalInput")
    wx_d = nc.dram_tensor("wx", [128, NWX, 128], bf16, kind="ExternalInput")
    wx8_d = nc.dram_tensor("wx8", [128, NWX, 128], fp8, kind="ExternalInput")
    wh8_d = nc.dram_tensor("wh8", [128, NWH, 128], fp8, kind="ExternalInput")
    b2_d = nc.dram_tensor("b2", [128, NJ], f32, kind="ExternalInput")
    bleaf_d = nc.dram_tensor("bleaf", [128, NJ], f32, kind="ExternalInput")
    b1_d = nc.dram_tensor("b1", [128, NJ], f32, kind="ExternalInput")

    c0t = nc.dram_tensor("c0t", [D, BC], f32, kind="ExternalOutput")
    h0t = nc.dram_tensor("h0t", [D, BC], f32, kind="ExternalOutput")

    xt8_r = xt8_d.ap()
    c0t_r = c0t.ap().rearrange("(kt p) b -> p kt b", p=128)
    h0t_r = h0t.ap().rearrange("(kt p) b -> p kt b", p=128)

    with tile.TileContext(nc) as tc:
        import contextlib

        ctx = contextlib.ExitStack()
        with ctx:
            wpool = ctx.enter_context(tc.tile_pool(name="wpool", bufs=1))
            hpool = ctx.enter_context(tc.tile_pool(name="hpool", bufs=1))
            inpool = ctx.enter_context(tc.tile_pool(name="inpool", bufs=2))
            gpool = ctx.enter_context(tc.tile_pool(name="gpool", bufs=1))
            epool = ctx.enter_context(tc.tile_pool(name="epool", bufs=1))
            pspool = ctx.enter_context(tc.tile_pool(name="ps", bufs=4, space="PSUM"))

            # --- weights / biases / tail inputs (one-time) ----------------
            wx_sb = wpool.tile([128, NWX, 128], bf16, name="wxsb")
            wx8_sb = wpool.tile([128, NWX, 128], fp8, name="wx8")
            wh8_sb = wpool.tile([128, NWH, 128], fp8, name="wh8")
            b2_sb = wpool.tile([128, NJ], f32, name="b2sb")
            bleaf_sb = wpool.tile([128, NJ], f32, name="bleafsb")
            b1_sb = wpool.tile([128, NJ], f32, name="b1sb")
            xtt_sb = wpool.tile([128, 32, 4, BC], bf16, name="xttsb")
            ident_sb = wpool.tile([128, 128], bf16, name="identsb")
            b2s_sb = wpool.tile([128, NJ], bf16, name="b2ssb")
            b1s_sb = wpool.tile([128, NJ], bf16, name="b1ssb")

            nc.sync.dma_start(out=bleaf_sb[:], in_=bleaf_d.ap()[:])
            nc.sync.dma_start(out=b2_sb[:], in_=b2_d.ap()[:])
            nc.sync.dma_start(out=b1_sb[:], in_=b1_d.ap()[:])
            nc.sync.dma_start(out=ident_sb[:], in_=ident_d.ap()[:])
            # x16-scaled bf16 biases for the identity-matmul bias injection
            nc.vector.tensor_single_scalar(
                b2s_sb[:], b2_sb[:], WSCALE, mybir.AluOpType.mult
            )
            nc.vector.tensor_single_scalar(
                b1s_sb[:], b1_sb[:], WSCALE, mybir.AluOpType.mult
            )
            # order: fp8 iou x-blocks (leaves first), h weights (127/L6),
            # fp8 fx blocks (L6/L5), bf16 wx + biases + tail x.
            for s in range(0, NWX_IOU, 12):
                nc.gpsimd.dma_start(
                    out=wx8_sb[:, s : s + 12, :], in_=wx8_d.ap()[:, s : s + 12, :]
                )
            nc.gpsimd.dma_start(out=xtt_sb[:], in_=xtt_d.ap()[:])
            for s in range(0, NWH, 32):
                nc.gpsimd.dma_start(
                    out=wh8_sb[:, s : s + 32, :], in_=wh8_d.ap()[:, s : s + 32, :]
                )
            nc.gpsimd.dma_start(
                out=wx8_sb[:, NWX_IOU:, :], in_=wx8_d.ap()[:, NWX_IOU:, :]
            )
            nc.gpsimd.dma_start(out=wx_sb[:], in_=wx_d.ap()[:])

            def process(
                nodes,
                has_l,
                has_r,
                bias_sb,
                child_h,  # list[(tile, base)] or None
                out_h,  # list[(tile, base)] or None (root)
                child_c=None,
                out_c=None,
                x8=False,
                bias_s_sb=None,
            ):
                a, b_ = nodes.start, nodes.stop
                K = b_ - a
                SUB = (K + 15) // 16
                ks = [min(16, K - 16 * s) for s in range(SUB)]
                to_out = out_h is None
                dt_g = f32 if to_out else bf16
                have_f = has_l or has_r

                # x input: fp8 streamed tile, or resident bf16 tail slice
                if x8:
                    xt_t = inpool.tile([128, K, 4, BC], fp8, name="xt8_t")
                    nc.sync.dma_start(out=xt_t[:], in_=xt8_r[:, a:b_, :, :])
                    xv = xt_t
                else:
                    p0 = TAIL_POS[a]
                    xv = xtt_sb[:, p0 : p0 + K, :, :]

                def x_insts(s, j):
                    n0 = 16 * s
                    n1 = n0 + ks[s]
                    jx = j - 4 if j >= 16 else j  # fR's x-part is fx too
                    if x8:
                        i0 = WX_IDX[(0, jx)]
                        return [
                            (
                                wx8_sb[:, i0 + kk : i0 + kk + 2, :],
                                xv[:, n0:n1, kk : kk + 2, :].rearrange(
                                    "p n kt b -> p kt n b"
                                ),
                                DR,
                            )
                            for kk in (0, 2)
                        ]
                    return [
                        (
                            wx_sb[:, WX_IDX[(kk, jx)], :],
                            xv[:, n0:n1, kk, :],
                            None,
                        )
                        for kk in range(4)
                    ]

                def h_insts(s, j):
                    if child_h is None:
                        return []
                    ch_base = child_h[0][1]
                    n0, n1 = a + 16 * s, a + 16 * s + ks[s]
                    sl0 = 2 * n0 + 1 - ch_base
                    kk = n1 - n0

                    def nsl(off):
                        s0 = sl0 + off
                        if kk == 1:
                            return slice(s0, s0 + 1)
                        return slice(s0, s0 + 2 * kk - 1, 2)

                    sides = []
                    if has_l and j < 16:
                        sides.append(("L", 0))
                    if has_r and (j < 12 or 16 <= j):
                        sides.append(("R", 1))
                    out = []
                    for side, off in sides:
                        i0 = WH_IDX[(side, 0, j)]
                        for ct, _ in child_h:
                            for kta in (0, 2):
                                out.append(
                                    (
                                        wh8_sb[:, i0 + kta : i0 + kta + 2, :],
                                        ct[:, nsl(off), kta : kta + 2, :].rearrange(
                                            "p n kt b -> p kt n b"
                                        ),
                                        DR,
                                    )
                                )
                    return out

                def mm_group(ps, j, x_part=True, h_part=True):
                    # x phase for all subs first, then h phase: stalled
                    # h-matmuls sit behind ready x-work, not in front of it
                    # (PE dependency wait-queue is only 4 deep). Each sub's
                    # region is its own bank, so per-sub start flags are safe.
                    phases = []
                    for s in range(SUB):
                        xi = x_insts(s, j) if x_part else []
                        hi = h_insts(s, j) if h_part else []
                        phases.append((s, xi, hi))
                    for pi in range(2):
                        for s, xi, hi in phases:
                            psv = ps[:, s, : ks[s], :]
                            insts = xi if pi == 0 else hi
                            if not insts:
                                continue
                            first = pi == 0 or not xi
                            last = pi == 1 or not hi
                            for m, (w_ap, rhs, pm) in enumerate(insts):
                                kw = {} if pm is None else {"perf_mode": pm}
                                nc.tensor.matmul(
                                    psv,
                                    w_ap,
                                    rhs,
                                    start=(first and m == 0),
                                    stop=(last and m == len(insts) - 1),
                                    **kw,
                                )

                g_i = gpool.tile([128, K, 4, BC], dt_g, name="g_i", bufs=2)
                g_o = gpool.tile([128, K, 4, BC], dt_g, name="g_o", bufs=2)
                g_u = gpool.tile([128, K, 4, BC], dt_g, name="g_u", bufs=2)
                if has_l:
                    g_fl = gpool.tile([128, K, 4, BC], dt_g, name="g_fl")
                if has_r:
                    g_fr = gpool.tile([128, K, 4, BC], dt_g, name="g_fr")

                def gv(t, sl):
                    """[p, SUB, 16, b] view of gate tile t's output-slice sl."""
                    if SUB == 1:
                        return t[:, :K, sl, :]
                    return t[:].rearrange("p (s n) kt b -> p s n kt b", s=SUB)[
                        :, :, :, sl, :
                    ]

                def psv_all(ps):
                    if SUB == 1:
                        return ps[:, 0, :K, :]
                    return ps[:]

                if out_c is not None:
                    oc_t, oc_base = out_c
                    c_t = oc_t[:, a - oc_base : b_ - oc_base, :, :]
                else:
                    c_t = epool.tile([128, K, 4, BC], dt_g, name="c_t")[:]
                if to_out:
                    h_t = epool.tile([128, K, 4, BC], dt_g, name="h_t")

                if child_c is not None:
                    cc_t, cc_base = child_c
                    cs0 = 2 * a + 1 - cc_base
                    if has_l:
                        if K == 1:
                            cl_t = cc_t[:, cs0 : cs0 + 1, :, :]
                        else:
                            cl_t = cc_t[:, cs0 : cs0 + 2 * K - 1 : 2, :, :]
                    if has_r:
                        if K == 1:
                            cr_t = cc_t[:, cs0 + 1 : cs0 + 2, :, :]
                        else:
                            cr_t = cc_t[:, cs0 + 1 : cs0 + 2 * K : 2, :, :]

                def gate_act(dst, sl, j, func):
                    ps = pspool.tile([128, 2, 16, BC], f32, name="ps")
                    mm_group(ps, j)
                    nc.scalar.activation(
                        out=gv(dst, sl),
                        in_=psv_all(ps),
                        func=func,
                        bias=bias_sb[:, j : j + 1],
                        scale=1.0 / WSCALE,
                    )

                if K <= 8:
                    # ---- gate-merged tail path: one act per gate, bias
                    # injected into PSUM via identity-matmul with a
                    # stride-0 broadcast rhs (bias pre-scaled x16).
                    # Two-phase emission: ready bias/x matmuls for several
                    # groups first, stalled h matmuls after, so the 4-deep
                    # PE wait-queue never hides ready work. start= is set
                    # only on the first matmul of each PSUM bank (slices
                    # share banks at small K; start clears the whole bank's
                    # has_written bits). -------------------------------------
                    spb = max(1, 512 // (K * BC))  # slices per psum bank

                    def mm_tail(ps, j0, phase, bias_mm=True, x_part=True,
                                h_part=True):
                        for sl in range(4):
                            j = j0 + sl
                            a_insts = []
                            if bias_mm:
                                a_insts.append(
                                    (
                                        ident_sb[:],
                                        bias_s_sb[:, j : j + 1].broadcast_to(
                                            [128, K * BC]
                                        ),
                                        None,
                                    )
                                )
                            if x_part:
                                a_insts += x_insts(0, j)
                            b_insts = h_insts(0, j) if h_part else []
                            insts = a_insts if phase == 0 else b_insts
                            if not insts:
                                continue
                            first_of_slice = phase == 0 or not a_insts
                            last_of_slice = phase == 1 or not b_insts
                            for m, (w_ap, rhs, pm) in enumerate(insts):
                                kw = {} if pm is None else {"perf_mode": pm}
                                st = (
                                    first_of_slice and m == 0 and sl % spb == 0
                                )
                                sp = last_of_slice and m == len(insts) - 1
                                nc.tensor.matmul(
                                    ps[:, sl, :, :], w_ap, rhs,
                                    start=st, stop=sp, **kw,
                                )

                    def gview(t):
                        return t[:, :K, :, :].rearrange("p n s b -> p s n b")

                    def ps4():
                        return pspool.tile([128, 4, K, BC], f32, name="ps")

                    ps_fl = ps4() if has_l else None
                    ps_fr = ps4() if has_r else None
                    ps_i, ps_u = ps4(), ps4()
                    # phase A: all ready (bias + x) work
                    if has_l:
                        mm_tail(ps_fl, 12, 0)
                    if has_r:
                        mm_tail(ps_fr, 16, 0)
                    mm_tail(ps_i, 0, 0)
                    mm_tail(ps_u, 8, 0)
                    # phase B: h accumulation
                    if has_l:
                        mm_tail(ps_fl, 12, 1)
                    if has_r:
                        mm_tail(ps_fr, 16, 1)
                    mm_tail(ps_i, 0, 1)
                    mm_tail(ps_u, 8, 1)
                    nc.scalar.activation(
                        out=gview(g_i), in_=ps_i[:], func=ACT.Sigmoid,
                        scale=1.0 / WSCALE,
                    )
                    nc.scalar.activation(
                        out=gview(g_u), in_=ps_u[:], func=ACT.Tanh,
                        scale=1.0 / WSCALE,
                    )
                    if has_l:
                        nc.scalar.activation(
                            out=gview(g_fl), in_=ps_fl[:],
                            func=ACT.Sigmoid, scale=1.0 / WSCALE,
                        )
                    if has_r:
                        nc.scalar.activation(
                            out=gview(g_fr), in_=ps_fr[:],
                            func=ACT.Sigmoid, scale=1.0 / WSCALE,
                        )
                    ps_o = ps4()
                    mm_tail(ps_o, 4, 0)
                    mm_tail(ps_o, 4, 1)
                    nc.scalar.activation(
                        out=gview(g_o), in_=ps_o[:], func=ACT.Sigmoid,
                        scale=1.0 / WSCALE,
                    )

                    gi = g_i[:, :K, :, :]
                    go = g_o[:, :K, :, :]
                    gu = g_u[:, :K, :, :]
                    nc.vector.tensor_mul(c_t, gi, gu)
                    if has_l:
                        nc.vector.tensor_mul(gi, g_fl[:, :K, :, :], cl_t)
                        nc.vector.tensor_add(c_t, c_t, gi)
                    if has_r:
                        nc.vector.tensor_mul(gi, g_fr[:, :K, :, :], cr_t)
                        nc.vector.tensor_add(c_t, c_t, gi)
                    nc.scalar.activation(out=gu, in_=c_t, func=ACT.Tanh)
                    if to_out:
                        nc.vector.tensor_mul(h_t[:], go, gu)
                        nc.sync.dma_start(out=c0t_r[:], in_=c_t[:, 0, :, :])
                        nc.sync.dma_start(out=h0t_r[:], in_=h_t[:, 0, :, :])
                    elif len(out_h) == 1:
                        oh_t, oh_base = out_h[0]
                        nc.vector.tensor_mul(
                            oh_t[:, a - oh_base : b_ - oh_base, :, :], go, gu
                        )
                    else:
                        hbf = g_fl[:, :K, :, :]
                        nc.vector.tensor_mul(hbf, go, gu)
                        hsl = slice(a - out_h[0][1], b_ - out_h[0][1])
                        h8s = out_h[0][0][:, hsl, :, :]
                        nc.vector.tensor_copy(h8s, hbf)
                        nc.vector.tensor_sub(out_h[1][0][:, hsl, :, :], hbf, h8s)
                    return

                # two ktpair halves: groups emitted in consumption order
                # {i,u} -> {fx} -> {fL,fR} -> {o}, then this half's
                # elementwise + tanh + h, so DoubleRow consumers of child h
                # at the next level unblock per-ktpair.
                for hp in (0, 2):
                    sls = (hp, hp + 1)
                    for sl in sls:
                        gate_act(g_i, sl, sl, ACT.Sigmoid)
                        gate_act(g_u, sl, 8 + sl, ACT.Tanh)
                    if have_f:
                        # fx recomputed into each side's psum group (x_part);
                        # act reads psum directly with the fused bias.
                        for side_j, g_f in (
                            (12, g_fl if has_l else None),
                            (16, g_fr if has_r else None),
                        ):
                            if g_f is None:
                                continue
                            for sl in sls:
                                gate_act(g_f, sl, side_j + sl, ACT.Sigmoid)
                    for sl in sls:
                        gate_act(g_o, sl, 4 + sl, ACT.Sigmoid)

                    # --- elementwise for this ktpair half ----------------
                    h2 = slice(hp, hp + 2)
                    ch = c_t[:, :, h2, :]
                    gi = g_i[:, :K, h2, :]
                    go = g_o[:, :K, h2, :]
                    gu = g_u[:, :K, h2, :]
                    nc.vector.tensor_mul(ch, gi, gu)
                    if has_l:
                        nc.vector.tensor_mul(gi, g_fl[:, :K, h2, :], cl_t[:, :, h2, :])
                        nc.vector.tensor_add(ch, ch, gi)
                    if has_r:
                        nc.vector.tensor_mul(gi, g_fr[:, :K, h2, :], cr_t[:, :, h2, :])
                        nc.vector.tensor_add(ch, ch, gi)
                    # tanh(c) -> reuse g_u
                    nc.scalar.activation(out=gu, in_=ch, func=ACT.Tanh)
                    if to_out:
                        nc.vector.tensor_mul(h_t[:, :, h2, :], go, gu)
                    elif len(out_h) == 1:
                        oh_t, oh_base = out_h[0]
                        nc.vector.tensor_mul(
                            oh_t[:, a - oh_base : b_ - oh_base, h2, :], go, gu
                        )
                    else:
                        # split-h: hbf reuses g_fl (consumed above)
                        hbf = g_fl[:, :K, h2, :]
                        nc.vector.tensor_mul(hbf, go, gu)
                        hsl = slice(a - out_h[0][1], b_ - out_h[0][1])
                        h8s = out_h[0][0][:, hsl, h2, :]
                        nc.vector.tensor_copy(h8s, hbf)
                        nc.vector.tensor_sub(
                            out_h[1][0][:, hsl, h2, :], hbf, h8s
                        )

                if to_out:
                    nc.sync.dma_start(out=c0t_r[:], in_=c_t[:, 0, :, :])
                    nc.sync.dma_start(out=h0t_r[:], in_=h_t[:, 0, :, :])

            # h storage: plain fp8 levels 4..7; split fp8 pair levels 1..3.
            # c: fp8 at level 7, bf16 below.
            H_SPLIT_LVLS = (3, 2, 1)

            for _rep in range(reps):
                leafc_h = hpool.tile([128, 129, 4, BC], fp8, name="h_leafc")
                leafc_c = hpool.tile([128, 129, 4, BC], fp8, name="c_leafc")
                lvl_h = {7: [(leafc_h, 127)]}
                lvl_c = {7: (leafc_c, 127)}
                for lvl in range(6, 0, -1):
                    base = 2**lvl - 1
                    if lvl in H_SPLIT_LVLS:
                        t8 = hpool.tile([128, 2**lvl, 4, BC], fp8, name=f"h_{lvl}")
                        r8 = hpool.tile([128, 2**lvl, 4, BC], fp8, name=f"hr_{lvl}")
                        lvl_h[lvl] = [(t8, base), (r8, base)]
                    else:
                        t = hpool.tile([128, 2**lvl, 4, BC], fp8, name=f"h_{lvl}")
                        lvl_h[lvl] = [(t, base)]
                    t = hpool.tile([128, 2**lvl, 4, BC], bf16, name=f"c_{lvl}")
                    lvl_c[lvl] = (t, base)

                # leaves in 32-node super-chunks; the one holding node 255
                # first so node 127's serial chain hides behind the rest.
                for s4 in (224, 128):
                    process(
                        range(s4, s4 + 32), False, False, bleaf_sb, None,
                        lvl_h[7], out_c=lvl_c[7], x8=True,
                    )
                    if s4 == 224:
                        process(
                            range(127, 128), True, False, b1_sb, lvl_h[7],
                            lvl_h[7], child_c=lvl_c[7], out_c=lvl_c[7],
                            bias_s_sb=b1s_sb,
                        )
                for s4 in (160, 192):
                    process(
                        range(s4, s4 + 32), False, False, bleaf_sb, None,
                        lvl_h[7], out_c=lvl_c[7], x8=True,
                    )
                # L6: B-half (63..94, needs node 127 + leaves 128..190) after
                # A-half? A (95..126) needs leaves 191..254 -> do B first?
                # B needs 127..190 (ready after leaves 128..191); A needs
                # 191..254 (ready after all leaves). Emit B then A.
                process(
                    range(63, 95), True, True, b2_sb, lvl_h[7], lvl_h[6],
                    child_c=lvl_c[7], out_c=lvl_c[6], x8=True,
                )
                process(
                    range(95, 127), True, True, b2_sb, lvl_h[7], lvl_h[6],
                    child_c=lvl_c[7], out_c=lvl_c[6], x8=True,
                )
                # L5 (one 32-node super-chunk), then L4..L1
                process(
                    range(31, 63), True, True, b2_sb, lvl_h[6], lvl_h[5],
                    child_c=lvl_c[6], out_c=lvl_c[5], x8=True,
                )
                # L4 as two 8-node gate-merged chunks (pipeline each other)
                for a4 in (15, 23):
                    process(
                        range(a4, a4 + 8), True, True, b2_sb, lvl_h[5],
                        lvl_h[4], child_c=lvl_c[5], out_c=lvl_c[4],
                        bias_s_sb=b2s_sb,
                    )
                for lvl in range(3, 0, -1):
                    process(
                        range(2**lvl - 1, 2 ** (lvl + 1) - 1), True, True,
                        b2_sb, lvl_h[lvl + 1], lvl_h[lvl],
                        child_c=lvl_c[lvl + 1], out_c=lvl_c[lvl],
                        bias_s_sb=b2s_sb,
                    )
                process(
                    range(0, 1), True, True, b2_sb, lvl_h[1], None,
                    child_c=lvl_c[1], bias_s_sb=b2s_sb,
                )

    nc.compile()
    return nc


def _expected_tree():
    left = np.array([2 * i + 1 if 2 * i + 1 < N else 0 for i in range(N)], np.int32)
    right = np.array([2 * i + 2 if 2 * i + 2 < N else 0 for i in range(N)], np.int32)
    nch = np.array(
        [int(2 * i + 1 < N) + int(2 * i + 2 < N) for i in range(N)], np.int32
    )
    return left, right, nch


def pack_w(W_ioux, W_fx, W_iouhL, W_fhL, W_iouhR, W_fhR):
    """Returns (wx bf16, wx8 fp8, wh8 fp8), partition-major [128, blk, 128]."""
    s = WSCALE
    WxT = np.asarray(W_ioux, np.float32).T * s
    WfxT = np.asarray(W_fx, np.float32).T * s
    wx = np.empty((NWX, 128, 128), np.float32)
    for i, (kt, j) in enumerate(W_X_BLOCKS):
        src = WxT if j < 12 else WfxT
        jj = j if j < 12 else j - 12
        wx[i] = src[kt * 128 : (kt + 1) * 128, jj * 128 : (jj + 1) * 128]

    WhT = {
        "L": (np.asarray(W_iouhL, np.float32).T * s,
              np.asarray(W_fhL, np.float32).T * s),
        "R": (np.asarray(W_iouhR, np.float32).T * s,
              np.asarray(W_fhR, np.float32).T * s),
    }
    wh = np.empty((NWH, 128, 128), np.float32)
    for i, (side, kt, j) in enumerate(W_H_BLOCKS):
        iou_m, f_m = WhT[side]
        if j < 12:
            wh[i] = iou_m[kt * 128 : (kt + 1) * 128, j * 128 : (j + 1) * 128]
        else:
            jj = (j - 12) if j < 16 else (j - 16)
            wh[i] = f_m[kt * 128 : (kt + 1) * 128, jj * 128 : (jj + 1) * 128]

    wx_pm = np.ascontiguousarray(wx.transpose(1, 0, 2))  # [128, blk, 128]
    wh_pm = np.ascontiguousarray(wh.transpose(1, 0, 2))
    return wx_pm.astype(BF16), wx_pm.astype(FP8), wh_pm.astype(FP8)


def pack_biases(b_ioux, b_iouh, b_iouhL, b_iouhR, b_fx, b_fhL, b_fhR):
    def pack(vec):
        return np.ascontiguousarray(np.asarray(vec, np.float32).reshape(NJ, 128).T)

    z = np.zeros(512, np.float32)
    b2 = pack(np.concatenate([b_ioux + b_iouhL + b_iouhR, b_fx + b_fhL, b_fx + b_fhR]))
    bleaf = pack(np.concatenate([b_ioux + b_iouh, z, z]))
    b1 = pack(np.concatenate([b_ioux + b_iouhL, b_fx + b_fhL, z]))
    return b2, bleaf, b1


def pack_x_all(inputs):
    """inputs: [B, N, D] f32 -> per-core (xt8 [128,N,4,BC] fp8,
    xtt [128,32,4,BC] bf16) lists, one vectorized pass."""
    x = inputs.reshape(NCORES, BC, N, 4, 128)
    xt = np.ascontiguousarray(x.transpose(0, 4, 2, 3, 1))  # [C,128,N,4,BC]
    xt8 = xt.astype(FP8)
    tail = np.empty((NCORES, 128, 32, 4, BC), np.float32)
    tail[:, :, :31] = xt[:, :, :31]
    tail[:, :, 31] = xt[:, :, 127]
    tail = tail.astype(BF16)
    return [xt8[c] for c in range(NCORES)], [tail[c] for c in range(NCORES)]


class _Runner:
    """jit once per nc; reuse the executable across calls."""

    def __init__(self, nc, n_cores):
        import jax
        from concourse import bass2jax
        from concourse.bass2jax import _bass_exec_p, install_neuronx_cc_hook

        install_neuronx_cc_hook()
        self.nc = nc
        self.n_cores = n_cores
        partition_name = (
            nc.partition_id_tensor.name if nc.partition_id_tensor else None
        )
        in_names, out_names, out_avals, zero_outs = [], [], [], []
        for alloc in nc.m.functions[0].allocations:
            if not isinstance(alloc, mybir.MemoryLocationSet):
                continue
            name = alloc.memorylocations[0].name
            if alloc.kind == "ExternalInput":
                if name != partition_name:
                    in_names.append(name)
            elif alloc.kind == "ExternalOutput":
                out_names.append(name)
                shape = tuple(alloc.tensor_shape)
                dtype = mybir.dt.np(alloc.dtype)
                out_avals.append(jax.core.ShapedArray(shape, dtype))
                zero_outs.append(np.zeros(shape, dtype))
        self.in_names = in_names
        self.out_names = out_names
        self.zero_outs = zero_outs
        n_params = len(in_names)
        all_in = in_names + out_names
        if partition_name is not None:
            all_in.append(partition_name)

        def _body(*args):
            operands = list(args)
            if partition_name is not None:
                operands.append(bass2jax.partition_id_tensor())
            outs = _bass_exec_p.bind(
                *operands,
                out_avals=tuple(out_avals),
                in_names=tuple(all_in),
                out_names=tuple(out_names),
                lowering_input_output_aliases=(),
                sim_require_finite=True,
                sim_require_nnan=True,
                nc=nc,
            )
            return tuple(outs)

        if n_cores == 1:
            self.fn = jax.jit(_body, keep_unused=True)
        else:
            from jax.sharding import Mesh, PartitionSpec
            from jax.experimental.shard_map import shard_map

            devices = jax.devices()[:n_cores]
            mesh = Mesh(np.asarray(devices), ("core",))
            n_out = len(out_names)
            self.fn = jax.jit(
                shard_map(
                    _body,
                    mesh=mesh,
                    in_specs=(PartitionSpec("core"),) * (n_params + n_out),
                    out_specs=(PartitionSpec("core"),) * n_out,
                    check_rep=False,
                ),
                keep_unused=True,
            )

    def __call__(self, in_maps):
        import jax

        n = self.n_cores
        if n == 1:
            args = [np.asarray(in_maps[0][k]) for k in self.in_names]
            args += [np.zeros_like(z) for z in self.zero_outs]
            outs = self.fn(*args)
            jax.block_until_ready(outs)
            return [{k: np.asarray(outs[i]) for i, k in enumerate(self.out_names)}]
        args = [
            np.concatenate([np.asarray(m[k]) for m in in_maps], axis=0)
            for k in self.in_names
        ]
        args += [
            np.zeros((n * z.shape[0], *z.shape[1:]), z.dtype) for z in self.zero_outs
        ]
        outs = self.fn(*args)
        jax.block_until_ready(outs)
        res = []
        for c in range(n):
            d = {}
            for i, k in enumerate(self.out_names):
                full = np.asarray(outs[i])
                per = full.shape[0] // n
                d[k] = full[c * per : (c + 1) * per]
            res.append(d)
        return res


def _make_in_maps(inputs, weights_args):
    wx, wx8, wh8 = pack_w(*weights_args[:6])
    b2, bleaf, b1 = pack_biases(*weights_args[6:])
    inputs = np.asarray(inputs, np.float32)
    ident = np.eye(128, dtype=BF16)
    xt8s, xtts = pack_x_all(inputs)
    in_maps = []
    for c in range(NCORES):
        in_maps.append(
            {"xt8": xt8s[c], "xtt": xtts[c], "wx": wx, "wx8": wx8, "wh8": wh8,
             "b2": b2, "bleaf": bleaf, "b1": b1, "ident": ident}
        )
    return in_maps


def kernel(
    inputs,
    W_ioux, b_ioux, W_iouh, b_iouh, W_iouhL, b_iouhL, W_iouhR, b_iouhR,
    W_fx, b_fx, W_fh, b_fh, W_fhL, b_fhL, W_fhR, b_fhR,
    left_idx, right_idx, num_children,
):
    el, er, en = _expected_tree()
    assert np.array_equal(np.asarray(left_idx), el), "unexpected tree structure"
    assert np.array_equal(np.asarray(right_idx), er), "unexpected tree structure"
    assert np.array_equal(np.asarray(num_children), en), "unexpected tree structure"

    weights_args = (W_ioux, W_fx, W_iouhL, W_fhL, W_iouhR, W_fhR,
                    b_ioux, b_iouh, b_iouhL, b_iouhR, b_fx, b_fhL, b_fhR)
    in_maps = _make_in_maps(inputs, weights_args)

    if "nc" not in _compiled:
        _compiled["nc"] = _build_bass()
    nc = _compiled["nc"]
    if "runner" not in _compiled:
        _compiled["runner"] = _Runner(nc, NCORES)
    res = _compiled["runner"](in_maps)
    _compiled["last_res"] = res

    c_full = np.empty((B, D), np.float32)
    h_full = np.empty((B, D), np.float32)
    for c in range(NCORES):
        c_full[c * BC : (c + 1) * BC] = res[c]["c0t"].T
        h_full[c * BC : (c + 1) * BC] = res[c]["h0t"].T
    return c_full, h_full


# revision 9
# speedup vs baseline: 3.3767x; 3.0154x over previous
"""ConstituencyTreeLSTM Trainium2 kernel, v2.

Changes vs v1 baseline:
  - Leaf x-path in fp8 DoubleRow (accuracy-validated: rel ~1.46e-2 < 2e-2).
  - 32-node super-chunks: per-j activations merged along the node axis
    (same output slice -> same bias), tanh(c)/h/elementwise merged across
    the 4 output slices. ~240 Act instructions instead of ~460.
  - fx recomputed into each f-gate side's PSUM group (A/B-benched faster
    on HW than sharing it via DVE copy+add, despite the extra matmuls).
  - Partition-major DRAM packing for weights and inputs: every DMA is
    contiguous per partition (KB-sized descriptor runs, not 32-64B).
  - Tail (nodes 0..30 + 127) x-inputs SBUF-resident, loaded once.
  - Tile reuse: tanh(c) overwrites g_u, mul scratch overwrites g_i,
    split-h intermediate overwrites g_fl.
"""

import sys

sys.path.insert(0, "/opt/trn_rl_repo")

import numpy as np
import ml_dtypes

import concourse.bass as bass  # noqa: F401
import concourse.mybir as mybir
import concourse.tile as tile
from concourse import bacc
from concourse.bass_utils import run_bass_kernel_spmd

BF16 = ml_dtypes.bfloat16
FP8 = ml_dtypes.float8_e4m3
NCORES = 8
B, N, D = 256, 256, 512
BC = B // NCORES
NJ = 20
WSCALE = 16.0

# x-path blocks: 12 iou j-tiles + 4 fx j-tiles, 4 k-tiles each; iou js
# cohort-ordered (j = co, 4+co, 8+co) so the first DMA piece covers the
# first j-groups processed.
W_X_BLOCKS = [
    (kt, j) for co in range(4) for j in (co, 4 + co, 8 + co) for kt in range(4)
] + [(kt, j) for j in range(12, 16) for kt in range(4)]
WX_IDX = {p: i for i, p in enumerate(W_X_BLOCKS)}
NWX = len(W_X_BLOCKS)  # 64
NWX_IOU = 48

W_H_BLOCKS = []
for j in range(12):
    W_H_BLOCKS += [("L", kt, j) for kt in range(4)]
    W_H_BLOCKS += [("R", kt, j) for kt in range(4)]
for j in range(12, 16):
    W_H_BLOCKS += [("L", kt, j) for kt in range(4)]
for j in range(16, 20):
    W_H_BLOCKS += [("R", kt, j) for kt in range(4)]
WH_IDX = {p: i for i, p in enumerate(W_H_BLOCKS)}
NWH = len(W_H_BLOCKS)  # 128

# tail nodes resident in SBUF: 0..30 plus 127 at position 31
TAIL_POS = {n: n for n in range(31)}
TAIL_POS[127] = 31

IOU_ORDER = [0, 4, 8, 1, 5, 9, 2, 6, 10, 3, 7, 11]  # cohort order (DMA-friendly)

_compiled = {}


def _build_bass(reps=1):
    nc = bacc.Bacc("TRN2", target_bir_lowering=False, debug=False, num_devices=NCORES)

    f32 = mybir.dt.float32
    bf16 = mybir.dt.bfloat16
    fp8 = mybir.dt.float8e4
    DR = mybir.MatmulPerfMode.DoubleRow
    ACT = mybir.ActivationFunctionType

    # partition-major DRAM layouts (host pre-packed)
    xt8_d = nc.dram_tensor("xt8", [128, N, 4, BC], fp8, kind="ExternalInput")
    xtt_d = nc.dram_tensor("xtt", [128, 32, 4, BC], bf16, kind="ExternalInput")
    ident_d = nc.dram_tensor("ident", [128, 128], bf16, kind="ExternalInput")
    wx_d = nc.dram_tensor("wx", [128, NWX, 128], bf16, kind="ExternalInput")
    wx8_d = nc.dram_tensor("wx8", [128, NWX, 128], fp8, kind="ExternalInput")
    wh8_d = nc.dram_tensor("wh8", [128, NWH, 128], fp8, kind="ExternalInput")
    b2_d = nc.dram_tensor("b2", [128, NJ], f32, kind="ExternalInput")
    bleaf_d = nc.dram_tensor("bleaf", [128, NJ], f32, kind="ExternalInput")
    b1_d = nc.dram_tensor("b1", [128, NJ], f32, kind="ExternalInput")

    c0t = nc.dram_tensor("c0t", [D, BC], f32, kind="ExternalOutput")
    h0t = nc.dram_tensor("h0t", [D, BC], f32, kind="ExternalOutput")

    xt8_r = xt8_d.ap()
    c0t_r = c0t.ap().rearrange("(kt p) b -> p kt b", p=128)
    h0t_r = h0t.ap().rearrange("(kt p) b -> p kt b", p=128)

    with tile.TileContext(nc) as tc:
        import contextlib

        ctx = contextlib.ExitStack()
        with ctx:
            wpool = ctx.enter_context(tc.tile_pool(name="wpool", bufs=1))
            hpool = ctx.enter_context(tc.tile_pool(name="hpool", bufs=1))
            inpool = ctx.enter_context(tc.tile_pool(name="inpool", bufs=2))
            gpool = ctx.enter_context(tc.tile_pool(name="gpool", bufs=1))
            epool = ctx.enter_context(tc.tile_pool(name="epool", bufs=1))
            pspool = ctx.enter_context(tc.tile_pool(name="ps", bufs=4, space="PSUM"))

            # --- weights / biases / tail inputs (one-time) ----------------
            wx_sb = wpool.tile([128, NWX, 128], bf16, name="wxsb")
            wx8_sb = wpool.tile([128, NWX, 128], fp8, name="wx8")
            wh8_sb = wpool.tile([128, NWH, 128], fp8, name="wh8")
            b2_sb = wpool.tile([128, NJ], f32, name="b2sb")
            bleaf_sb = wpool.tile([128, NJ], f32, name="bleafsb")
            b1_sb = wpool.tile([128, NJ], f32, name="b1sb")
            xtt_sb = wpool.tile([128, 32, 4, BC], bf16, name="xttsb")
            ident_sb = wpool.tile([128, 128], bf16, name="identsb")
            b2s_sb = wpool.tile([128, NJ], bf16, name="b2ssb")
            b1s_sb = wpool.tile([128, NJ], bf16, name="b1ssb")

            nc.sync.dma_start(out=bleaf_sb[:], in_=bleaf_d.ap()[:])
            nc.sync.dma_start(out=b2_sb[:], in_=b2_d.ap()[:])
            nc.sync.dma_start(out=b1_sb[:], in_=b1_d.ap()[:])
            nc.sync.dma_start(out=ident_sb[:], in_=ident_d.ap()[:])
            # x16-scaled bf16 biases for the identity-matmul bias injection
            nc.vector.tensor_single_scalar(
                b2s_sb[:], b2_sb[:], WSCALE, mybir.AluOpType.mult
            )
            nc.vector.tensor_single_scalar(
                b1s_sb[:], b1_sb[:], WSCALE, mybir.AluOpType.mult
            )
            # order: fp8 iou x-blocks (leaves first), h weights (127/L6),
            # fp8 fx blocks (L6/L5), bf16 wx + biases + tail x.
            for s in range(0, NWX_IOU, 12):
                nc.gpsimd.dma_start(
                    out=wx8_sb[:, s : s + 12, :], in_=wx8_d.ap()[:, s : s + 12, :]
                )
            nc.gpsimd.dma_start(out=xtt_sb[:], in_=xtt_d.ap()[:])
            for s in range(0, NWH, 32):
                nc.gpsimd.dma_start(
                    out=wh8_sb[:, s : s + 32, :], in_=wh8_d.ap()[:, s : s + 32, :]
                )
            nc.gpsimd.dma_start(
                out=wx8_sb[:, NWX_IOU:, :], in_=wx8_d.ap()[:, NWX_IOU:, :]
            )
            nc.gpsimd.dma_start(out=wx_sb[:], in_=wx_d.ap()[:])

            def process(
                nodes,
                has_l,
                has_r,
                bias_sb,
                child_h,  # list[(tile, base)] or None
                out_h,  # list[(tile, base)] or None (root)
                child_c=None,
                out_c=None,
                x8=False,
                bias_s_sb=None,
            ):
                a, b_ = nodes.start, nodes.stop
                K = b_ - a
                SUB = (K + 15) // 16
                ks = [min(16, K - 16 * s) for s in range(SUB)]
                to_out = out_h is None
                dt_g = f32 if to_out else bf16
                have_f = has_l or has_r

                # x input: fp8 streamed tile, or resident bf16 tail slice
                if x8:
                    xt_t = inpool.tile([128, K, 4, BC], fp8, name="xt8_t")
                    nc.sync.dma_start(out=xt_t[:], in_=xt8_r[:, a:b_, :, :])
                    xv = xt_t
                else:
                    p0 = TAIL_POS[a]
                    xv = xtt_sb[:, p0 : p0 + K, :, :]

                def x_insts(s, j):
                    n0 = 16 * s
                    n1 = n0 + ks[s]
                    jx = j - 4 if j >= 16 else j  # fR's x-part is fx too
                    if x8:
                        i0 = WX_IDX[(0, jx)]
                        return [
                            (
                                wx8_sb[:, i0 + kk : i0 + kk + 2, :],
                                xv[:, n0:n1, kk : kk + 2, :].rearrange(
                                    "p n kt b -> p kt n b"
                                ),
                                DR,
                            )
                            for kk in (0, 2)
                        ]
                    return [
                        (
                            wx_sb[:, WX_IDX[(kk, jx)], :],
                            xv[:, n0:n1, kk, :],
                            None,
                        )
                        for kk in range(4)
                    ]

                def h_insts(s, j):
                    if child_h is None:
                        return []
                    ch_base = child_h[0][1]
                    n0, n1 = a + 16 * s, a + 16 * s + ks[s]
                    sl0 = 2 * n0 + 1 - ch_base
                    kk = n1 - n0

                    def nsl(off):
                        s0 = sl0 + off
                        if kk == 1:
                            return slice(s0, s0 + 1)
                        return slice(s0, s0 + 2 * kk - 1, 2)

                    sides = []
                    if has_l and j < 16:
                        sides.append(("L", 0))
                    if has_r and (j < 12 or 16 <= j):
                        sides.append(("R", 1))
                    out = []
                    for side, off in sides:
                        i0 = WH_IDX[(side, 0, j)]
                        for ct, _ in child_h:
                            for kta in (0, 2):
                                out.append(
                                    (
                                        wh8_sb[:, i0 + kta : i0 + kta + 2, :],
                                        ct[:, nsl(off), kta : kta + 2, :].rearrange(
                                            "p n kt b -> p kt n b"
                                        ),
                                        DR,
                                    )
                                )
                    return out

                def mm_group(ps, j, x_part=True, h_part=True):
                    # x phase for all subs first, then h phase: stalled
                    # h-matmuls sit behind ready x-work, not in front of it
                    # (PE dependency wait-queue is only 4 deep). Each sub's
                    # region is its own bank, so per-sub start flags are safe.
                    phases = []
                    for s in range(SUB):
                        xi = x_insts(s, j) if x_part else []
                        hi = h_insts(s, j) if h_part else []
                        phases.append((s, xi, hi))
                    for pi in range(2):
                        for s, xi, hi in phases:
                            psv = ps[:, s, : ks[s], :]
                            insts = xi if pi == 0 else hi
                            if not insts:
                                continue
                            first = pi == 0 or not xi
                            last = pi == 1 or not hi
                            for m, (w_ap, rhs, pm) in enumerate(insts):
                                kw = {} if pm is None else {"perf_mode": pm}
                                nc.tensor.matmul(
                                    psv,
                                    w_ap,
                                    rhs,
                                    start=(first and m == 0),
                                    stop=(last and m == len(insts) - 1),
                                    **kw,
                                )

                g_i = gpool.tile([128, K, 4, BC], dt_g, name="g_i", bufs=2)
                g_o = gpool.tile([128, K, 4, BC], dt_g, name="g_o", bufs=2)
                g_u = gpool.tile([128, K, 4, BC], dt_g, name="g_u", bufs=2)
                if has_l:
                    g_fl = gpool.tile([128, K, 4, BC], dt_g, name="g_fl")
                if has_r:
                    g_fr = gpool.tile([128, K, 4, BC], dt_g, name="g_fr")

                def gv(t, sl):
                    """[p, SUB, 16, b] view of gate tile t's output-slice sl."""
                    if SUB == 1:
                        return t[:, :K, sl, :]
                    return t[:].rearrange("p (s n) kt b -> p s n kt b", s=SUB)[
                        :, :, :, sl, :
                    ]

                def psv_all(ps):
                    if SUB == 1:
                        return ps[:, 0, :K, :]
                    return ps[:]

                if out_c is not None:
                    oc_t, oc_base = out_c
                    c_t = oc_t[:, a - oc_base : b_ - oc_base, :, :]
                else:
                    c_t = epool.tile([128, K, 4, BC], dt_g, name="c_t")[:]
                if to_out:
                    h_t = epool.tile([128, K, 4, BC], dt_g, name="h_t")

                if child_c is not None:
                    cc_t, cc_base = child_c
                    cs0 = 2 * a + 1 - cc_base
                    if has_l:
                        if K == 1:
                            cl_t = cc_t[:, cs0 : cs0 + 1, :, :]
                        else:
                            cl_t = cc_t[:, cs0 : cs0 + 2 * K - 1 : 2, :, :]
                    if has_r:
                        if K == 1:
                            cr_t = cc_t[:, cs0 + 1 : cs0 + 2, :, :]
                        else:
                            cr_t = cc_t[:, cs0 + 1 : cs0 + 2 * K : 2, :, :]

                def gate_act(dst, sl, j, func):
                    ps = pspool.tile([128, 2, 16, BC], f32, name="ps")
                    mm_group(ps, j)
                    nc.scalar.activation(
                        out=gv(dst, sl),
                        in_=psv_all(ps),
                        func=func,
                        bias=bias_sb[:, j : j + 1],
                        scale=1.0 / WSCALE,
                    )

                if K <= 8:
                    # ---- gate-merged tail path: one act per gate, bias
                    # injected into PSUM via identity-matmul with a
                    # stride-0 broadcast rhs (bias pre-scaled x16).
                    # Two-phase emission: ready bias/x matmuls for several
                    # groups first, stalled h matmuls after, so the 4-deep
                    # PE wait-queue never hides ready work. start= is set
                    # only on the first matmul of each PSUM bank (slices
                    # share banks at small K; start clears the whole bank's
                    # has_written bits). -------------------------------------
                    spb = max(1, 512 // (K * BC))  # slices per psum bank

                    def mm_tail(ps, j0, phase, bias_mm=True, x_part=True,
                                h_part=True):
                        for sl in range(4):
                            j = j0 + sl
                            a_insts = []
                            if bias_mm:
                                a_insts.append(
                                    (
                                        ident_sb[:],
                                        bias_s_sb[:, j : j + 1].broadcast_to(
                                            [128, K * BC]
                                        ),
                                        None,
                                    )
                                )
                            if x_part:
                                a_insts += x_insts(0, j)
                            b_insts = h_insts(0, j) if h_part else []
                            insts = a_insts if phase == 0 else b_insts
                            if not insts:
                                continue
                            first_of_slice = phase == 0 or not a_insts
                            last_of_slice = phase == 1 or not b_insts
                            for m, (w_ap, rhs, pm) in enumerate(insts):
                                kw = {} if pm is None else {"perf_mode": pm}
                                st = (
                                    first_of_slice and m == 0 and sl % spb == 0
                                )
                                sp = last_of_slice and m == len(insts) - 1
                                nc.tensor.matmul(
                                    ps[:, sl, :, :], w_ap, rhs,
                                    start=st, stop=sp, **kw,
                                )

                    def gview(t):
                        return t[:, :K, :, :].rearrange("p n s b -> p s n b")

                    def ps4():
                        return pspool.tile([128, 4, K, BC], f32, name="ps")

                    ps_fl = ps4() if has_l else None
                    ps_fr = ps4() if has_r else None
                    ps_i, ps_u = ps4(), ps4()
                    # phase A: all ready (bias + x) work
                    if has_l:
                        mm_tail(ps_fl, 12, 0)
                    if has_r:
                        mm_tail(ps_fr, 16, 0)
                    mm_tail(ps_i, 0, 0)
                    mm_tail(ps_u, 8, 0)
                    # phase B: h accumulation
                    if has_l:
                        mm_tail(ps_fl, 12, 1)
                    if has_r:
                        mm_tail(ps_fr, 16, 1)
                    mm_tail(ps_i, 0, 1)
                    mm_tail(ps_u, 8, 1)
                    nc.scalar.activation(
                        out=gview(g_i), in_=ps_i[:], func=ACT.Sigmoid,
                        scale=1.0 / WSCALE,
                    )
                    nc.scalar.activation(
                        out=gview(g_u), in_=ps_u[:], func=ACT.Tanh,
                        scale=1.0 / WSCALE,
                    )
                    if has_l:
                        nc.scalar.activation(
                            out=gview(g_fl), in_=ps_fl[:],
                            func=ACT.Sigmoid, scale=1.0 / WSCALE,
                        )
                    if has_r:
                        nc.scalar.activation(
                            out=gview(g_fr), in_=ps_fr[:],
                            func=ACT.Sigmoid, scale=1.0 / WSCALE,
                        )
                    ps_o = ps4()
                    mm_tail(ps_o, 4, 0)
                    mm_tail(ps_o, 4, 1)
                    nc.scalar.activation(
                        out=gview(g_o), in_=ps_o[:], func=ACT.Sigmoid,
                        scale=1.0 / WSCALE,
                    )

                    # elementwise + tanh + h per ktpair half: h01 releases
                    # early for the next level's first DoubleRow consumers
                    for h2 in (slice(0, 2), slice(2, 4)):
                        ch = c_t[:, :, h2, :]
                        gi = g_i[:, :K, h2, :]
                        go = g_o[:, :K, h2, :]
                        gu = g_u[:, :K, h2, :]
                        nc.vector.tensor_mul(ch, gi, gu)
                        if has_l:
                            nc.vector.tensor_mul(
                                gi, g_fl[:, :K, h2, :], cl_t[:, :, h2, :]
                            )
                            nc.vector.tensor_add(ch, ch, gi)
                        if has_r:
                            nc.vector.tensor_mul(
                                gi, g_fr[:, :K, h2, :], cr_t[:, :, h2, :]
                            )
                            nc.vector.tensor_add(ch, ch, gi)
                        nc.scalar.activation(out=gu, in_=ch, func=ACT.Tanh)
                        if to_out:
                            nc.vector.tensor_mul(h_t[:, :, h2, :], go, gu)
                        elif len(out_h) == 1:
                            oh_t, oh_base = out_h[0]
                            nc.vector.tensor_mul(
                                oh_t[:, a - oh_base : b_ - oh_base, h2, :], go, gu
                            )
                        else:
                            hbf = g_fl[:, :K, h2, :]
                            nc.vector.tensor_mul(hbf, go, gu)
                            hsl = slice(a - out_h[0][1], b_ - out_h[0][1])
                            h8s = out_h[0][0][:, hsl, h2, :]
                            nc.vector.tensor_copy(h8s, hbf)
                            nc.vector.tensor_sub(
                                out_h[1][0][:, hsl, h2, :], hbf, h8s
                            )
                    if to_out:
                        nc.sync.dma_start(out=c0t_r[:], in_=c_t[:, 0, :, :])
                        nc.sync.dma_start(out=h0t_r[:], in_=h_t[:, 0, :, :])
                    return

                # two ktpair halves: groups emitted in consumption order
                # {i,u} -> {fx} -> {fL,fR} -> {o}, then this half's
                # elementwise + tanh + h, so DoubleRow consumers of child h
                # at the next level unblock per-ktpair.
                for hp in (0, 2):
                    sls = (hp, hp + 1)
                    for sl in sls:
                        gate_act(g_i, sl, sl, ACT.Sigmoid)
                        gate_act(g_u, sl, 8 + sl, ACT.Tanh)
                    if have_f:
                        # fx recomputed into each side's psum group (x_part);
                        # act reads psum directly with the fused bias.
                        for side_j, g_f in (
                            (12, g_fl if has_l else None),
                            (16, g_fr if has_r else None),
                        ):
                            if g_f is None:
                                continue
                            for sl in sls:
                                gate_act(g_f, sl, side_j + sl, ACT.Sigmoid)
                    for sl in sls:
                        gate_act(g_o, sl, 4 + sl, ACT.Sigmoid)

                    # --- elementwise for this ktpair half ----------------
                    h2 = slice(hp, hp + 2)
                    ch = c_t[:, :, h2, :]
                    gi = g_i[:, :K, h2, :]
                    go = g_o[:, :K, h2, :]
                    gu = g_u[:, :K, h2, :]
                    nc.vector.tensor_mul(ch, gi, gu)
                    if has_l:
                        nc.vector.tensor_mul(gi, g_fl[:, :K, h2, :], cl_t[:, :, h2, :])
                        nc.vector.tensor_add(ch, ch, gi)
                    if has_r:
                        nc.vector.tensor_mul(gi, g_fr[:, :K, h2, :], cr_t[:, :, h2, :])
                        nc.vector.tensor_add(ch, ch, gi)
                    # tanh(c) -> reuse g_u
                    nc.scalar.activation(out=gu, in_=ch, func=ACT.Tanh)
                    if to_out:
                        nc.vector.tensor_mul(h_t[:, :, h2, :], go, gu)
                    elif len(out_h) == 1:
                        oh_t, oh_base = out_h[0]
                        nc.vector.tensor_mul(
                            oh_t[:, a - oh_base : b_ - oh_base, h2, :], go, gu
                        )
                    else:
                        # split-h: hbf reuses g_fl (consumed above)
                        hbf = g_fl[:, :K, h2, :]
                        nc.vector.tensor_mul(hbf, go, gu)
                        hsl = slice(a - out_h[0][1], b_ - out_h[0][1])
                        h8s = out_h[0][0][:, hsl, h2, :]
                        nc.vector.tensor_copy(h8s, hbf)
                        nc.vector.tensor_sub(
                            out_h[1][0][:, hsl, h2, :], hbf, h8s
                        )

                if to_out:
                    nc.sync.dma_start(out=c0t_r[:], in_=c_t[:, 0, :, :])
                    nc.sync.dma_start(out=h0t_r[:], in_=h_t[:, 0, :, :])

            # h storage: plain fp8 levels 4..7; split fp8 pair levels 1..3.
            # c: fp8 at level 7, bf16 below.
            H_SPLIT_LVLS = (3, 2, 1)

            for _rep in range(reps):
                leafc_h = hpool.tile([128, 129, 4, BC], fp8, name="h_leafc")
                leafc_c = hpool.tile([128, 129, 4, BC], fp8, name="c_leafc")
                lvl_h = {7: [(leafc_h, 127)]}
                lvl_c = {7: (leafc_c, 127)}
                for lvl in range(6, 0, -1):
                    base = 2**lvl - 1
                    if lvl in H_SPLIT_LVLS:
                        t8 = hpool.tile([128, 2**lvl, 4, BC], fp8, name=f"h_{lvl}")
                        r8 = hpool.tile([128, 2**lvl, 4, BC], fp8, name=f"hr_{lvl}")
                        lvl_h[lvl] = [(t8, base), (r8, base)]
                    else:
                        t = hpool.tile([128, 2**lvl, 4, BC], fp8, name=f"h_{lvl}")
                        lvl_h[lvl] = [(t, base)]
                    t = hpool.tile([128, 2**lvl, 4, BC], bf16, name=f"c_{lvl}")
                    lvl_c[lvl] = (t, base)

                # leaves in 32-node super-chunks; the one holding node 255
                # first so node 127's serial chain hides behind the rest.
                for s4 in (224, 128):
                    process(
                        range(s4, s4 + 32), False, False, bleaf_sb, None,
                        lvl_h[7], out_c=lvl_c[7], x8=True,
                    )
                    if s4 == 224:
                        process(
                            range(127, 128), True, False, b1_sb, lvl_h[7],
                            lvl_h[7], child_c=lvl_c[7], out_c=lvl_c[7],
                            bias_s_sb=b1s_sb,
                        )
                for s4 in (160, 192):
                    process(
                        range(s4, s4 + 32), False, False, bleaf_sb, None,
                        lvl_h[7], out_c=lvl_c[7], x8=True,
                    )
                # L6: B-half (63..94, needs node 127 + leaves 128..190) after
                # A-half? A (95..126) needs leaves 191..254 -> do B first?
                # B needs 127..190 (ready after leaves 128..191); A needs
                # 191..254 (ready after all leaves). Emit B then A.
                process(
                    range(63, 95), True, True, b2_sb, lvl_h[7], lvl_h[6],
                    child_c=lvl_c[7], out_c=lvl_c[6], x8=True,
                )
                process(
                    range(95, 127), True, True, b2_sb, lvl_h[7], lvl_h[6],
                    child_c=lvl_c[7], out_c=lvl_c[6], x8=True,
                )
                # L5 (one 32-node super-chunk), then L4..L1
                process(
                    range(31, 63), True, True, b2_sb, lvl_h[6], lvl_h[5],
                    child_c=lvl_c[6], out_c=lvl_c[5], x8=True,
                )
                # L4 as two 8-node gate-merged chunks (pipeline each other)
                for a4 in (15, 23):
                    process(
                        range(a4, a4 + 8), True, True, b2_sb, lvl_h[5],
                        lvl_h[4], child_c=lvl_c[5], out_c=lvl_c[4],
                        bias_s_sb=b2s_sb,
                    )
                for lvl in range(3, 0, -1):
                    process(
                        range(2**lvl - 1, 2 ** (lvl + 1) - 1), True, True,
                        b2_sb, lvl_h[lvl + 1], lvl_h[lvl],
                        child_c=lvl_c[lvl + 1], out_c=lvl_c[lvl],
                        bias_s_sb=b2s_sb,
                    )
                process(
                    range(0, 1), True, True, b2_sb, lvl_h[1], None,
                    child_c=lvl_c[1], bias_s_sb=b2s_sb,
                )

    nc.compile()
    return nc


def _expected_tree():
    left = np.array([2 * i + 1 if 2 * i + 1 < N else 0 for i in range(N)], np.int32)
    right = np.array([2 * i + 2 if 2 * i + 2 < N else 0 for i in range(N)], np.int32)
    nch = np.array(
        [int(2 * i + 1 < N) + int(2 * i + 2 < N) for i in range(N)], np.int32
    )
    return left, right, nch


def pack_w(W_ioux, W_fx, W_iouhL, W_fhL, W_iouhR, W_fhR):
    """Returns (wx bf16, wx8 fp8, wh8 fp8), partition-major [128, blk, 128]."""
    s = WSCALE
    WxT = np.asarray(W_ioux, np.float32).T * s
    WfxT = np.asarray(W_fx, np.float32).T * s
    wx = np.empty((NWX, 128, 128), np.float32)
    for i, (kt, j) in enumerate(W_X_BLOCKS):
        src = WxT if j < 12 else WfxT
        jj = j if j < 12 else j - 12
        wx[i] = src[kt * 128 : (kt + 1) * 128, jj * 128 : (jj + 1) * 128]

    WhT = {
        "L": (np.asarray(W_iouhL, np.float32).T * s,
              np.asarray(W_fhL, np.float32).T * s),
        "R": (np.asarray(W_iouhR, np.float32).T * s,
              np.asarray(W_fhR, np.float32).T * s),
    }
    wh = np.empty((NWH, 128, 128), np.float32)
    for i, (side, kt, j) in enumerate(W_H_BLOCKS):
        iou_m, f_m = WhT[side]
        if j < 12:
            wh[i] = iou_m[kt * 128 : (kt + 1) * 128, j * 128 : (j + 1) * 128]
        else:
            jj = (j - 12) if j < 16 else (j - 16)
            wh[i] = f_m[kt * 128 : (kt + 1) * 128, jj * 128 : (jj + 1) * 128]

    wx_pm = np.ascontiguousarray(wx.transpose(1, 0, 2))  # [128, blk, 128]
    wh_pm = np.ascontiguousarray(wh.transpose(1, 0, 2))
    return wx_pm.astype(BF16), wx_pm.astype(FP8), wh_pm.astype(FP8)


def pack_biases(b_ioux, b_iouh, b_iouhL, b_iouhR, b_fx, b_fhL, b_fhR):
    def pack(vec):
        return np.ascontiguousarray(np.asarray(vec, np.float32).reshape(NJ, 128).T)

    z = np.zeros(512, np.float32)
    b2 = pack(np.concatenate([b_ioux + b_iouhL + b_iouhR, b_fx + b_fhL, b_fx + b_fhR]))
    bleaf = pack(np.concatenate([b_ioux + b_iouh, z, z]))
    b1 = pack(np.concatenate([b_ioux + b_iouhL, b_fx + b_fhL, z]))
    return b2, bleaf, b1


def pack_x_all(inputs):
    """inputs: [B, N, D] f32 -> per-core (xt8 [128,N,4,BC] fp8,
    xtt [128,32,4,BC] bf16) lists, one vectorized pass."""
    x = inputs.reshape(NCORES, BC, N, 4, 128)
    xt = np.ascontiguousarray(x.transpose(0, 4, 2, 3, 1))  # [C,128,N,4,BC]
    xt8 = xt.astype(FP8)
    tail = np.empty((NCORES, 128, 32, 4, BC), np.float32)
    tail[:, :, :31] = xt[:, :, :31]
    tail[:, :, 31] = xt[:, :, 127]
    tail = tail.astype(BF16)
    return [xt8[c] for c in range(NCORES)], [tail[c] for c in range(NCORES)]


class _Runner:
    """jit once per nc; reuse the executable across calls."""

    def __init__(self, nc, n_cores):
        import jax
        from concourse import bass2jax
        from concourse.bass2jax import _bass_exec_p, install_neuronx_cc_hook

        install_neuronx_cc_hook()
        self.nc = nc
        self.n_cores = n_cores
        partition_name = (
            nc.partition_id_tensor.name if nc.partition_id_tensor else None
        )
        in_names, out_names, out_avals, zero_outs = [], [], [], []
        for alloc in nc.m.functions[0].allocations:
            if not isinstance(alloc, mybir.MemoryLocationSet):
                continue
            name = alloc.memorylocations[0].name
            if alloc.kind == "ExternalInput":
                if name != partition_name:
                    in_names.append(name)
            elif alloc.kind == "ExternalOutput":
                out_names.append(name)
                shape = tuple(alloc.tensor_shape)
                dtype = mybir.dt.np(alloc.dtype)
                out_avals.append(jax.core.ShapedArray(shape, dtype))
                zero_outs.append(np.zeros(shape, dtype))
        self.in_names = in_names
        self.out_names = out_names
        self.zero_outs = zero_outs
        n_params = len(in_names)
        all_in = in_names + out_names
        if partition_name is not None:
            all_in.append(partition_name)

        def _body(*args):
            operands = list(args)
            if partition_name is not None:
                operands.append(bass2jax.partition_id_tensor())
            outs = _bass_exec_p.bind(
                *operands,
                out_avals=tuple(out_avals),
                in_names=tuple(all_in),
                out_names=tuple(out_names),
                lowering_input_output_aliases=(),
                sim_require_finite=True,
                sim_require_nnan=True,
                nc=nc,
            )
            return tuple(outs)

        if n_cores == 1:
            self.fn = jax.jit(_body, keep_unused=True)
        else:
            from jax.sharding import Mesh, PartitionSpec
            from jax.experimental.shard_map import shard_map

            devices = jax.devices()[:n_cores]
            mesh = Mesh(np.asarray(devices), ("core",))
            n_out = len(out_names)
            self.fn = jax.jit(
                shard_map(
                    _body,
                    mesh=mesh,
                    in_specs=(PartitionSpec("core"),) * (n_params + n_out),
                    out_specs=(PartitionSpec("core"),) * n_out,
                    check_rep=False,
                ),
                keep_unused=True,
            )

    def __call__(self, in_maps):
        import jax

        n = self.n_cores
        if n == 1:
            args = [np.asarray(in_maps[0][k]) for k in self.in_names]
            args += [np.zeros_like(z) for z in self.zero_outs]
            outs = self.fn(*args)
            jax.block_until_ready(outs)
            return [{k: np.asarray(outs[i]) for i, k in enumerate(self.out_names)}]
        args = [
            np.concatenate([np.asarray(m[k]) for m in in_maps], axis=0)
            for k in self.in_names
        ]
        args += [
            np.zeros((n * z.shape[0], *z.shape[1:]), z.dtype) for z in self.zero_outs
        ]
        outs = self.fn(*args)
        jax.block_until_ready(outs)
        res = []
        for c in range(n):
            d = {}
            for i, k in enumerate(self.out_names):
                full = np.asarray(outs[i])
                per = full.shape[0] // n
                d[k] = full[c * per : (c + 1) * per]
            res.append(d)
        return res


def _make_in_maps(inputs, weights_args):
    wx, wx8, wh8 = pack_w(*weights_args[:6])
    b2, bleaf, b1 = pack_biases(*weights_args[6:])
    inputs = np.asarray(inputs, np.float32)
    ident = np.eye(128, dtype=BF16)
    xt8s, xtts = pack_x_all(inputs)
    in_maps = []
    for c in range(NCORES):
        in_maps.append(
            {"xt8": xt8s[c], "xtt": xtts[c], "wx": wx, "wx8": wx8, "wh8": wh8,
             "b2": b2, "bleaf": bleaf, "b1": b1, "ident": ident}
        )
    return in_maps


def kernel(
    inputs,
    W_ioux, b_ioux, W_iouh, b_iouh, W_iouhL, b_iouhL, W_iouhR, b_iouhR,
    W_fx, b_fx, W_fh, b_fh, W_fhL, b_fhL, W_fhR, b_fhR,
    left_idx, right_idx, num_children,
):
    el, er, en = _expected_tree()
    assert np.array_equal(np.asarray(left_idx), el), "unexpected tree structure"
    assert np.array_equal(np.asarray(right_idx), er), "unexpected tree structure"
    assert np.array_equal(np.asarray(num_children), en), "unexpected tree structure"

    weights_args = (W_ioux, W_fx, W_iouhL, W_fhL, W_iouhR, W_fhR,
                    b_ioux, b_iouh, b_iouhL, b_iouhR, b_fx, b_fhL, b_fhR)
    in_maps = _make_in_maps(inputs, weights_args)

    if "nc" not in _compiled:
        _compiled["nc"] = _build_bass()
    nc = _compiled["nc"]
    if "runner" not in _compiled:
        _compiled["runner"] = _Runner(nc, NCORES)
    res = _compiled["runner"](in_maps)
    _compiled["last_res"] = res

    c_full = np.empty((B, D), np.float32)
    h_full = np.empty((B, D), np.float32)
    for c in range(NCORES):
        c_full[c * BC : (c + 1) * BC] = res[c]["c0t"].T
        h_full[c * BC : (c + 1) * BC] = res[c]["h0t"].T
    return c_full, h_full
